# revision 18
# baseline (speedup 1.0000x reference)
"""Photonic transformer on 8 Trainium2 NeuronCores.

Sharding: tensor-parallel attention (1 head per core, all 16 wavelengths
local), token-parallel FFN (two 64-token half-shards per core), vocab-parallel
output projection (4000 cols per core). Comm per layer: per-batch
ReduceScatter of (out-proj partials + x/8) and per-batch AllGather of the
updated residual half — batch 0's RS/FFN/AG overlaps batch 1's attention, and
the per-batch-half LN1/QKV lets layer l+1's batch-0 attention overlap layer
l's tail (cross-layer pipeline). The wavelength softmax uses one ACT exp per
wavelength (per-partition scale = cos^2(phase)/sqrt(64), free-axis accum_out
for row sums) and one fused scalar_tensor_tensor accumulate per wavelength
(split DVE/GPSIMD); the summed attention matrix hits the AV matmul once. The
Kerr factor cos(2.7e-12*s^2) == 1.0 in fp32 and is dropped. LayerNorm affine
params are folded into the following weights on the host.
"""
import os
import numpy as np
import ml_dtypes

N_CORES = 8
B, S, D, H, HEAD, W, L, VOCAB = 2, 512, 512, 8, 64, 16, 6, 32000
T = B * S              # 1024 tokens
NT = T // 128          # 8 token tiles
KT = D // 128          # 4 k-tiles over D
FF = 4 * D             # 2048
KF = FF // 128         # 16 k-tiles over FF
VS = VOCAB // N_CORES  # 4000 vocab cols per core
NCHV = 8
VCH = VS // NCHV       # 500
LN_EPS = 1e-5

# wavelength indices whose accumulate runs on gpsimd (empirical knob)
GPS_W = ()
USE_TREE = os.environ.get("PK_TREE", "1") == "1"
L_RUN = int(os.environ.get("PK_LAYERS", L))
USE_COLL = os.environ.get("PK_COLL", "1") == "1"
DO_VOCAB = os.environ.get("PK_VOCAB", "1") == "1"

_CACHE = {}


def _build_nc():
    from concourse import bacc, mybir, tile
    from concourse.bass import IndirectOffsetOnAxis

    F32, BF16, I32 = mybir.dt.float32, mybir.dt.bfloat16, mybir.dt.int32
    AF, ALU = mybir.ActivationFunctionType, mybir.AluOpType

    nc = bacc.Bacc(None, num_devices=N_CORES)

    # ---------------- I/O ----------------
    ids_d = nc.dram_tensor("ids", [128, NT], I32, kind="ExternalInput")
    temb_d = nc.dram_tensor("temb", [VOCAB, D], F32, kind="ExternalInput")
    pos_d = nc.dram_tensor("pos", [S, D], F32, kind="ExternalInput")
    ident_d = nc.dram_tensor("ident", [128, 128], BF16, kind="ExternalInput")
    lw = []
    for l in range(L):
        lw.append(dict(
            qkvw=nc.dram_tensor(f"qkvw{l}", [KT, 128, 3 * HEAD], BF16, kind="ExternalInput"),
            qkvb=nc.dram_tensor(f"qkvb{l}", [1, 3 * HEAD], BF16, kind="ExternalInput"),
            ow=nc.dram_tensor(f"ow{l}", [HEAD, D], BF16, kind="ExternalInput"),
            ob8=nc.dram_tensor(f"ob8{l}", [1, D], BF16, kind="ExternalInput"),
            c2s=nc.dram_tensor(f"c2s{l}", [128, W], F32, kind="ExternalInput"),
            iw16=nc.dram_tensor(f"iw16{l}", [128, W], F32, kind="ExternalInput"),
            f1w=nc.dram_tensor(f"f1w{l}", [KT, 128, FF], BF16, kind="ExternalInput"),
            f1b=nc.dram_tensor(f"f1b{l}", [1, FF], BF16, kind="ExternalInput"),
            f2w=nc.dram_tensor(f"f2w{l}", [KF, 128, D], BF16, kind="ExternalInput"),
            f2b=nc.dram_tensor(f"f2b{l}", [1, D], BF16, kind="ExternalInput"),
        ))
    outw_d = nc.dram_tensor("outw", [KT, 128, VS], BF16, kind="ExternalInput")
    outb_d = nc.dram_tensor("outb", [1, VS], BF16, kind="ExternalInput")
    out_d = nc.dram_tensor("out", [T, VS], F32, kind="ExternalOutput")

    with tile.TileContext(nc) as tc:
        with (
            tc.tile_pool(name="const", bufs=1) as cpool,
            tc.tile_pool(name="wq", bufs=2) as wq,        # per-layer small weights
            tc.tile_pool(name="wf", bufs=1) as wf,        # per-layer FFN weights
            tc.tile_pool(name="xres", bufs=2) as xres,    # residual stream (x)
            tc.tile_pool(name="xp", bufs=1) as xp,        # per-layer persistents
            tc.tile_pool(name="act", bufs=2) as ap,       # activations
            tc.tile_pool(name="ep", bufs=2) as ep,        # exp tiles
            tc.tile_pool(name="sm", bufs=2) as sm,        # small stats
            tc.tile_pool(name="dram", bufs=2, space="DRAM") as dp,
            tc.tile_pool(name="ps_sc", bufs=1, space="PSUM") as ps_sc,
            tc.tile_pool(name="ps_mm", bufs=2, space="PSUM") as ps_mm,
            tc.tile_pool(name="ps_tp", bufs=2, space="PSUM") as ps_tp,
            tc.tile_pool(name="ps_h1", bufs=1, space="PSUM") as ps_h1,
        ):
            ones = cpool.tile([128, T], BF16)
            nc.gpsimd.memset(ones[:], 1.0)
            ident = cpool.tile([128, 128], BF16)
            nc.sync.dma_start(ident[:], ident_d[:])
            ids_sb = cpool.tile([128, NT], I32)
            nc.sync.dma_start(ids_sb[:], ids_d[:])
            pos_sb = ep.tile([128, NT // 2, D], F32, tag="E")
            for t in range(NT // 2):
                nc.sync.dma_start(pos_sb[:, t, :], pos_d[t * 128:(t + 1) * 128, :])

            # ---------------- embedding ----------------
            x_sb = xres.tile([128, NT, D], F32, tag="x")
            gath = ep.tile([128, NT, D], F32, tag="E")
            for t in range(NT):
                nc.gpsimd.indirect_dma_start(
                    out=gath[:, t, :], out_offset=None,
                    in_=temb_d[:, :],
                    in_offset=IndirectOffsetOnAxis(ap=ids_sb[:, t:t + 1], axis=0),
                )
            for t in range(NT):
                nc.vector.tensor_tensor(x_sb[:, t, :], gath[:, t, :],
                                        pos_sb[:, t % (NT // 2), :], ALU.add)

            def quake_rstd(veps, n, rows=128):
                """veps [rows, n] f32 AP (var+eps) -> rstd tile, quake + 3 Newton."""
                y0i_t = sm.tile([128, n], I32, tag="qk_i", name="qk_i")
                y0i = y0i_t[0:rows]
                nc.vector.tensor_scalar(y0i, veps.bitcast(I32), 1, None,
                                        op0=ALU.logical_shift_right)
                nc.vector.tensor_scalar(y0i, y0i, 0x5F3759DF, -1,
                                        op0=ALU.subtract, op1=ALU.mult)
                yt = sm.tile([128, n], F32, tag="qk_y", name="qk_y")
                y = yt[0:rows]
                nc.vector.tensor_copy(y, y0i.bitcast(F32))
                t1_t = sm.tile([128, n], F32, tag="qk_t", name="qk_t")
                t1 = t1_t[0:rows]
                for _ in range(3):
                    nc.vector.tensor_tensor(t1, y, y, ALU.mult)
                    nc.vector.tensor_tensor(t1, t1, veps, ALU.mult)
                    nc.vector.tensor_scalar(t1, t1, -0.5, 1.5, op0=ALU.mult, op1=ALU.add)
                    nc.vector.tensor_tensor(y, y, t1, ALU.mult)
                return yt

            # ================ layers ================
            for l in range(L_RUN):
                w = lw[l]
                qkvw = wq.tile([128, KT, 3 * HEAD], BF16, tag="qkvw")
                qkvb = wq.tile([128, 3 * HEAD], BF16, tag="qkvb")
                ow_sb = wq.tile([128, D], BF16, tag="ow")
                ob8 = wq.tile([128, D], BF16, tag="ob8")
                c2s = wq.tile([128, W], F32, tag="c2s")
                iw16 = wq.tile([128, W], F32, tag="iw16")
                nc.sync.dma_start(qkvw[:], w["qkvw"][:].rearrange("k p c -> p k c"))
                nc.sync.dma_start(qkvb[0:1, :], w["qkvb"][:])
                nc.sync.dma_start(ow_sb[0:HEAD, :], w["ow"][:])
                nc.sync.dma_start(ob8[0:1, :], w["ob8"][:])
                nc.sync.dma_start(c2s[:], w["c2s"][:])
                nc.sync.dma_start(iw16[:], w["iw16"][:])
                f1w = wf.tile([128, KT, FF], BF16, tag="f1w")
                f1b = wf.tile([128, FF], BF16, tag="f1b")
                f2w = wf.tile([128, KF, D], BF16, tag="f2w")
                f2b = wf.tile([128, D], BF16, tag="f2b")
                nc.sync.dma_start(f1w[:], w["f1w"][:].rearrange("k p c -> p k c"))
                nc.sync.dma_start(f1b[0:1, :], w["f1b"][:])
                nc.sync.dma_start(f2w[:], w["f2w"][:].rearrange("k p c -> p k c"))
                nc.sync.dma_start(f2b[0:1, :], w["f2b"][:])

                zT = xp.tile([128, KT, T], BF16, tag="zT")
                qT = ap.tile([128, T], BF16, tag="qT")
                kTt = ap.tile([128, T], BF16, tag="kTt")
                v_sb = xp.tile([128, NT, HEAD], BF16, tag="v_sb")
                OT_sb = xp.tile([128, NT, 128], BF16, tag="OT")
                x_next = xres.tile([128, NT, D], F32, tag="x", name="x_next")

                def ffn_half(rs_out_b):
                    """LN2 + photonic FFN on this core's 64-token half."""
                    x1 = ap.tile([128, D], F32, tag="x1h", name="x1h")
                    nc.sync.dma_start(x1[0:64, :], rs_out_b[:])
                    st6b = sm.tile([128, 1, 6], F32, tag="st6b", name="st6b")
                    mvb = sm.tile([128, 1, 2], F32, tag="mvb", name="mvb")
                    nc.vector.bn_stats(st6b[0:64, 0, :], x1[0:64, :])
                    nc.vector.bn_aggr(mvb[0:64, 0, :], st6b[0:64, 0, :])
                    negmu2 = sm.tile([128, 1], F32, tag="negmu2", name="negmu2")
                    veps2 = sm.tile([128, 1], F32, tag="veps2", name="veps2")
                    nc.vector.tensor_scalar_mul(negmu2[0:64], mvb[0:64, :, 0], -1.0)
                    nc.vector.tensor_scalar_add(veps2[0:64], mvb[0:64, :, 1], LN_EPS)
                    rstd2 = quake_rstd(veps2[0:64], 1, rows=64)
                    z2 = ap.tile([128, D], BF16, tag="z2", name="z2")
                    nc.vector.tensor_scalar(z2[0:64, :], x1[0:64, :],
                                            negmu2[0:64, 0:1], rstd2[0:64, 0:1],
                                            op0=ALU.add, op1=ALU.mult)
                    z2T = ap.tile([128, KT, 64], BF16, tag="z2T", name="z2T")
                    tp = ps_tp.tile([128, KT, 128], BF16, tag="tp", name="tp")
                    for k in range(KT):
                        nc.tensor.matmul(tp[:, k, 0:64], z2[0:64, k * 128:(k + 1) * 128],
                                         ident[0:64, 0:64], is_transpose=True,
                                         start=(k == 0), stop=(k == KT - 1))
                    nc.vector.tensor_copy(z2T[:], tp[:, :, 0:64])
                    actF = ap.tile([128, KF, 64], BF16, tag="actF", name="actF")
                    for hf in range(2):
                        h1 = ps_h1.tile([128, 2, 512], F32, tag="h1", name="h1")
                        for nch in range(2):
                            ncol = hf * 1024 + nch * 512
                            for k in range(KT):
                                nc.tensor.matmul(h1[0:64, nch, :], z2T[:, k, :],
                                                 f1w[:, k, ncol:ncol + 512],
                                                 start=(k == 0), stop=False)
                            nc.tensor.matmul(h1[0:64, nch, :], ones[0:1, 0:64],
                                             f1b[0:1, ncol:ncol + 512], start=False, stop=True)
                        for nch in range(2):
                            tnh = ap.tile([128, 512], BF16, tag="tnh", name="tnh")
                            nc.scalar.activation(tnh[0:64, :], h1[0:64, nch, :], AF.Tanh,
                                                 scale=2.0)
                            den = ap.tile([128, 512], F32, tag="den", name="den")
                            nc.scalar.activation(den[0:64, :], h1[0:64, nch, :], AF.Abs)
                            nc.vector.tensor_scalar_add(den[0:64, :], den[0:64, :], 1.0)
                            nc.vector.reciprocal_approx_fast(den[0:64, :], den[0:64, :])
                            af = ap.tile([128, 512], BF16, tag="af", name="af")
                            nc.vector.scalar_tensor_tensor(af[0:64, :], den[0:64, :], 1.0,
                                                           tnh[0:64, :],
                                                           op0=ALU.mult, op1=ALU.mult)
                            tp = ps_tp.tile([128, KT, 128], BF16, tag="tp", name="tp")
                            for j in range(4):
                                nc.tensor.matmul(tp[:, j, 0:64],
                                                 af[0:64, j * 128:(j + 1) * 128],
                                                 ident[0:64, 0:64], is_transpose=True,
                                                 start=(j == 0), stop=(j == 3))
                            nc.vector.tensor_copy(
                                actF[:, hf * 8 + nch * 4:hf * 8 + nch * 4 + 4, :],
                                tp[:, :, 0:64])
                    x2 = ps_mm.tile([128, D], F32, tag="mm", name="x2")
                    for kk in range(KF):
                        nc.tensor.matmul(x2[0:64, :], actF[:, kk, :], f2w[:, kk, :],
                                         start=(kk == 0), stop=False)
                    nc.tensor.matmul(x2[0:64, :], ones[0:1, 0:64], f2b[0:1, :],
                                     start=False, stop=True)
                    y_sb = ap.tile([128, D], F32, tag="y_sb", name="y_sb")
                    nc.vector.scalar_tensor_tensor(y_sb[0:64, :], x1[0:64, :], 1.0,
                                                   x2[0:64, :], op0=ALU.mult, op1=ALU.add)
                    ag_in = dp.tile([64, D], F32, tag="ag_in", name="ag_in")
                    nc.sync.dma_start(ag_in[:], y_sb[0:64, :])
                    return ag_in

                for b in range(B):
                    bsl = slice(b * 512, (b + 1) * 512)
                    # ---- LN1 + z^T + QKV for this batch half
                    st6 = sm.tile([128, 4, 6], F32, tag="st6", name="st6")
                    mv = sm.tile([128, 4, 2], F32, tag="mv", name="mv")
                    for q in range(4):
                        t = b * 4 + q
                        nc.vector.bn_stats(st6[:, q, :], x_sb[:, t, :])
                        nc.vector.bn_aggr(mv[:, q, :], st6[:, q, :])
                    negmu = sm.tile([128, 4], F32, tag="negmu", name="negmu")
                    veps = sm.tile([128, 4], F32, tag="veps", name="veps")
                    nc.vector.tensor_scalar_mul(negmu[:], mv[:, :, 0], -1.0)
                    nc.vector.tensor_scalar_add(veps[:], mv[:, :, 1], LN_EPS)
                    rstd = quake_rstd(veps[:], 4)
                    for q in range(4):
                        t = b * 4 + q
                        z = ap.tile([128, D], BF16, tag="z", name="z")
                        nc.vector.tensor_scalar(z[:], x_sb[:, t, :],
                                                negmu[:, q:q + 1], rstd[:, q:q + 1],
                                                op0=ALU.add, op1=ALU.mult)
                        tp = ps_tp.tile([128, KT, 128], BF16, tag="tp", name="tp")
                        for k in range(KT):
                            nc.tensor.matmul(tp[:, k, :], z[:, k * 128:(k + 1) * 128],
                                             ident[:], is_transpose=True,
                                             start=(k == 0), stop=(k == KT - 1))
                        nc.vector.tensor_copy(zT[:, :, t * 128:(t + 1) * 128], tp[:])
                    for which, dst in ((0, qT), (1, kTt)):
                        ps_full = ps_mm.tile([128, 512], F32, tag="mm", name="qk_ps")
                        ps = ps_full[0:HEAD, :]
                        cs = slice(which * HEAD, (which + 1) * HEAD)
                        for k in range(KT):
                            nc.tensor.matmul(ps, qkvw[:, k, cs], zT[:, k, bsl],
                                             start=(k == 0), stop=False)
                        nc.tensor.matmul(ps, qkvb[0:1, cs], ones[0:1, bsl],
                                         start=False, stop=True)
                        nc.vector.tensor_copy(dst[0:HEAD, bsl], ps)
                    for q in range(4):
                        t = b * 4 + q
                        ps = ps_mm.tile([128, HEAD], F32, tag="mm", name="v_ps")
                        for k in range(KT):
                            nc.tensor.matmul(ps[:], zT[:, k, t * 128:(t + 1) * 128],
                                             qkvw[:, k, 2 * HEAD:3 * HEAD],
                                             start=(k == 0), stop=False)
                        nc.tensor.matmul(ps[:], ones[0:1, :128],
                                         qkvb[0:1, 2 * HEAD:3 * HEAD], start=False, stop=True)
                        nc.vector.tensor_copy(v_sb[:, t, :], ps[:])

                    # ---- attention over the batch
                    rs_in_b = dp.tile([S, D], F32, tag="rs_in", name="rs_in")
                    AT = ap.tile([128, KT, 512], BF16, tag="AT", name="AT")
                    ot_b = ps_h1.tile([128, 512], F32, tag="otb", name="ot_b")
                    for st in range(4):
                        sc = ps_sc.tile([128, 512], F32, tag="sc", name="sc")
                        nc.tensor.matmul(
                            sc[:], qT[0:HEAD, b * 512 + st * 128: b * 512 + (st + 1) * 128],
                            kTt[0:HEAD, bsl])
                        E = ep.tile([128, W, 512], BF16, tag="E", name="E")
                        Z = sm.tile([128, W], F32, tag="Z", name="Z")
                        for wl in range(W):
                            nc.scalar.activation(E[:, wl, :], sc[:], AF.Exp,
                                                 scale=c2s[:, wl:wl + 1],
                                                 accum_out=Z[:, wl:wl + 1])
                        R = sm.tile([128, W], F32, tag="R", name="R")
                        nc.vector.reciprocal_approx_fast(R[:], Z[:])
                        nc.vector.tensor_tensor(R[:], R[:], iw16[:], ALU.mult)
                        A = ap.tile([128, 512], BF16, tag="A", name="A")
                        if USE_TREE:
                            # P_w = E_w * r_w in place, then pairwise tree adds
                            for wl in range(W):
                                nc.vector.tensor_scalar_mul(E[:, wl, :], E[:, wl, :],
                                                            R[:, wl:wl + 1])
                            nc.vector.tensor_tensor(E[:, 0:8, :], E[:, 0:8, :],
                                                    E[:, 8:16, :], ALU.add)
                            nc.vector.tensor_tensor(E[:, 0:4, :], E[:, 0:4, :],
                                                    E[:, 4:8, :], ALU.add)
                            nc.vector.tensor_tensor(E[:, 0:2, :], E[:, 0:2, :],
                                                    E[:, 2:4, :], ALU.add)
                            nc.vector.tensor_tensor(A[:], E[:, 0, :], E[:, 1, :], ALU.add)
                        else:
                            nc.vector.tensor_scalar_mul(A[:], E[:, 0, :], R[:, 0:1])
                            for wl in range(1, W):
                                nc.vector.scalar_tensor_tensor(
                                    A[:], E[:, wl, :], R[:, wl:wl + 1], A[:],
                                    op0=ALU.mult, op1=ALU.add)
                        tp = ps_tp.tile([128, KT, 128], BF16, tag="tp", name="tp")
                        for j in range(KT):
                            nc.tensor.matmul(tp[:, j, :], A[:, j * 128:(j + 1) * 128],
                                             ident[:], is_transpose=True,
                                             start=(j == 0), stop=(j == KT - 1))
                        nc.vector.tensor_copy(AT[:, :, st * 128:(st + 1) * 128], tp[:])
                        for j in range(KT):
                            nc.tensor.matmul(ot_b[0:HEAD, st * 128:(st + 1) * 128],
                                             v_sb[:, b * 4 + j, :],
                                             AT[:, j, st * 128:(st + 1) * 128],
                                             start=(st == 0 and j == 0),
                                             stop=(st == 3 and j == KT - 1))
                    nc.vector.tensor_copy(OT_sb[0:HEAD, b * 4:(b + 1) * 4, :],
                                          ot_b[0:HEAD, :].rearrange("p (s c) -> p s c", c=128))
                    for st in range(4):
                        tok = b * 4 + st
                        pp = ps_mm.tile([128, D], F32, tag="mm", name="pp")
                        nc.tensor.matmul(pp[:], OT_sb[0:HEAD, tok, :], ow_sb[0:HEAD, :],
                                         start=True, stop=False)
                        nc.tensor.matmul(pp[:], ones[0:1, :128], ob8[0:1, :],
                                         start=False, stop=True)
                        p_sb = ap.tile([128, D], F32, tag="p_sb", name="p_sb")
                        nc.vector.scalar_tensor_tensor(p_sb[:], x_sb[:, tok, :], 0.125,
                                                       pp[:], op0=ALU.mult, op1=ALU.add)
                        nc.sync.dma_start(rs_in_b[st * 128:(st + 1) * 128, :], p_sb[:])

                    # ---- per-batch ReduceScatter -> half FFN -> per-batch AllGather
                    rs_out_b = dp.tile([64, D], F32, tag="rs_out", name="rs_out")
                    if USE_COLL:
                        nc.gpsimd.collective_compute(
                            "ReduceScatter", ALU.add,
                            replica_groups=[list(range(N_CORES))],
                            ins=[rs_in_b[:].opt()], outs=[rs_out_b[:].opt()],
                        )
                    else:
                        nc.sync.dma_start(rs_out_b[:], rs_in_b[0:64, :])
                    ag_in_b = ffn_half(rs_out_b)
                    ag_out_b = dp.tile([S, D], F32, tag="ag_out", name="ag_out",
                                       addr_space="Shared")
                    if USE_COLL:
                        nc.gpsimd.collective_compute(
                            "AllGather", ALU.bypass,
                            replica_groups=[list(range(N_CORES))],
                            ins=[ag_in_b[:].opt()], outs=[ag_out_b[:].opt()],
                        )
                    else:
                        for _r in range(N_CORES):
                            nc.sync.dma_start(ag_out_b[_r * 64:(_r + 1) * 64, :], ag_in_b[:])
                    # next-layer residual stream, this batch's 4 token tiles
                    nc.sync.dma_start(
                        x_next[:, b * 4:(b + 1) * 4, :],
                        ag_out_b[:].rearrange("(q p2 o) d -> (p2 o) q d", q=4, p2=2, o=64))

                x_sb = x_next

            # ================ vocab projection ================
            if not DO_VOCAB:
                fin = ap.tile([128, 8], F32, tag="fin")
                nc.vector.tensor_copy(fin[:], x_sb[:, 0, 0:8])
                nc.sync.dma_start(out_d[0:128, 0:8], fin[:])
            xvT = xp.tile([128, KT, T], BF16, tag="zT", name="xvT")
            for t in (range(NT) if DO_VOCAB else []):
                xc = ap.tile([128, D], BF16, tag="z", name="xc")
                nc.vector.tensor_copy(xc[:], x_sb[:, t, :])
                tp = ps_tp.tile([128, KT, 128], BF16, tag="tp", name="tp")
                for k in range(KT):
                    nc.tensor.matmul(tp[:, k, :], xc[:, k * 128:(k + 1) * 128], ident[:],
                                     is_transpose=True, start=(k == 0), stop=(k == KT - 1))
                nc.vector.tensor_copy(xvT[:, :, t * 128:(t + 1) * 128], tp[:])
            for nch in (range(NCHV) if DO_VOCAB else []):
                ow_c = wq.tile([128, KT, VCH], BF16, tag="outwc", name="ow_c")
                nc.sync.dma_start(
                    ow_c[:], outw_d[:, :, nch * VCH:(nch + 1) * VCH].rearrange("k p c -> p k c"))
                ob_c = wq.tile([128, VCH], BF16, tag="outbc", name="ob_c")
                nc.sync.dma_start(ob_c[0:1, :], outb_d[:, nch * VCH:(nch + 1) * VCH])
                for m in range(NT):
                    vp = ps_mm.tile([128, VCH], F32, tag="mm", name="vp")
                    for k in range(KT):
                        nc.tensor.matmul(vp[:], xvT[:, k, m * 128:(m + 1) * 128],
                                         ow_c[:, k, :],
                                         start=(k == 0), stop=False)
                    nc.tensor.matmul(vp[:], ones[0:1, :128], ob_c[0:1, :],
                                     start=False, stop=True)
                    ob = ap.tile([128, VCH], F32, tag="ob", name="ob")
                    if (m * NCHV + nch) % 2 == 0:
                        nc.vector.tensor_copy(ob[:], vp[:])
                    else:
                        nc.scalar.copy(ob[:], vp[:])
                    nc.sync.dma_start(out_d[m * 128:(m + 1) * 128, nch * VCH:(nch + 1) * VCH],
                                      ob[:])

    nc.compile()
    return nc


def prep_inputs(input_ids, token_emb, pos_emb, layers, out_w, out_b):
    """Host-side input marshalling: per-core weight slices, LN folding, layout."""
    bf16 = ml_dtypes.bfloat16
    f32 = np.float32
    ids = np.asarray(input_ids).reshape(-1).astype(np.int32)       # [1024]
    ids_pt = ids.reshape(NT, 128).T.copy()                          # [128, NT]
    temb = np.asarray(token_emb, f32)
    pos = np.asarray(pos_emb, f32)[:S]
    ident = np.eye(128, dtype=bf16)
    out_w = np.asarray(out_w, f32)
    out_b = np.asarray(out_b, f32)

    in_maps = []
    for r in range(N_CORES):
        m = dict(ids=ids_pt, temb=temb, pos=pos, ident=ident)
        for l, p in enumerate(layers):
            qw = np.asarray(p["qw"], f32) * np.asarray(p["ln1g"], f32)[:, None]
            kw = np.asarray(p["kw"], f32) * np.asarray(p["ln1g"], f32)[:, None]
            vw = np.asarray(p["vw"], f32) * np.asarray(p["ln1g"], f32)[:, None]
            qb = np.asarray(p["qb"], f32) + np.asarray(p["ln1b"], f32) @ np.asarray(p["qw"], f32)
            kb = np.asarray(p["kb"], f32) + np.asarray(p["ln1b"], f32) @ np.asarray(p["kw"], f32)
            vb = np.asarray(p["vb"], f32) + np.asarray(p["ln1b"], f32) @ np.asarray(p["vw"], f32)
            hsl = slice(r * HEAD, (r + 1) * HEAD)
            qkvw = np.concatenate([qw[:, hsl], kw[:, hsl], vw[:, hsl]], axis=1)  # [512,192]
            m[f"qkvw{l}"] = qkvw.astype(bf16).reshape(KT, 128, 3 * HEAD)
            m[f"qkvb{l}"] = np.concatenate([qb[hsl], kb[hsl], vb[hsl]])[None, :].astype(bf16)
            m[f"ow{l}"] = np.asarray(p["ow"], f32)[hsl, :].astype(bf16)
            m[f"ob8{l}"] = (np.asarray(p["ob"], f32) / N_CORES)[None, :].astype(bf16)
            c2 = (np.cos(np.asarray(p["phase"], f32)[:, r]) ** 2) * np.float32(1.0 / np.sqrt(HEAD))
            m[f"c2s{l}"] = np.tile(c2[None, :], (128, 1)).astype(f32)
            iw = np.asarray(p["iw"], f32) / np.float32(W)
            m[f"iw16{l}"] = np.tile(iw[None, :], (128, 1)).astype(f32)
            f1w = np.asarray(p["f1w"], f32) * np.asarray(p["ln2g"], f32)[:, None]
            f1b = np.asarray(p["f1b"], f32) + np.asarray(p["ln2b"], f32) @ np.asarray(p["f1w"], f32)
            m[f"f1w{l}"] = f1w.astype(bf16).reshape(KT, 128, FF)
            m[f"f1b{l}"] = f1b[None, :].astype(bf16)
            m[f"f2w{l}"] = np.asarray(p["f2w"], f32).astype(bf16).reshape(KF, 128, D)
            m[f"f2b{l}"] = np.asarray(p["f2b"], f32)[None, :].astype(bf16)
        vsl = slice(r * VS, (r + 1) * VS)
        m["outw"] = out_w[:, vsl].astype(bf16).reshape(KT, 128, VS)
        m["outb"] = out_b[vsl][None, :].astype(bf16)
        in_maps.append(m)
    return in_maps


def _get_exec():
    if "exec" in _CACHE:
        return _CACHE["exec"]
    import jax
    from jax.sharding import Mesh, PartitionSpec
    from jax.experimental.shard_map import shard_map
    from concourse import bass2jax, mybir

    nc = _build_nc()
    bass2jax.install_neuronx_cc_hook()
    partition_name = nc.partition_id_tensor.name if nc.partition_id_tensor else None
    in_names, out_names, out_avals, zero_outs = [], [], [], []
    for alloc in nc.m.functions[0].allocations:
        if not isinstance(alloc, mybir.MemoryLocationSet):
            continue
        name = alloc.memorylocations[0].name
        if alloc.kind == "ExternalInput":
            if name != partition_name:
                in_names.append(name)
        elif alloc.kind == "ExternalOutput":
            out_names.append(name)
            shape = tuple(alloc.tensor_shape)
            dt = mybir.dt.np(alloc.dtype)
            out_avals.append(jax.core.ShapedArray(shape, dt))
            zero_outs.append(np.zeros(shape, dt))
    n_params = len(in_names)
    all_names = list(in_names) + list(out_names) + ([partition_name] if partition_name else [])

    def _body(*args):
        operands = list(args)
        if partition_name is not None:
            operands.append(bass2jax.partition_id_tensor())
        outs = bass2jax._bass_exec_p.bind(
            *operands,
            out_avals=tuple(out_avals),
            in_names=tuple(all_names),
            out_names=tuple(out_names),
            lowering_input_output_aliases=(),
            sim_require_finite=True,
            sim_require_nnan=True,
            nc=nc,
        )
        return tuple(outs)

    devices = jax.devices()[:N_CORES]
    mesh = Mesh(np.asarray(devices), ("core",))
    in_specs = (PartitionSpec("core"),) * (n_params + len(out_names))
    out_specs = (PartitionSpec("core"),) * len(out_names)
    fn = jax.jit(
        shard_map(_body, mesh=mesh, in_specs=in_specs, out_specs=out_specs,
                  check_rep=False),
        keep_unused=True,
    )
    _CACHE["exec"] = (fn, in_names, out_names, out_avals, zero_outs)
    return _CACHE["exec"]


def _device_args(in_maps):
    import jax
    fn, in_names, out_names, out_avals, zero_outs = _get_exec()
    concat_in = [
        np.concatenate([np.asarray(in_maps[c][n]) for c in range(N_CORES)], axis=0)
        for n in in_names
    ]
    concat_zero = [
        np.zeros((N_CORES * z.shape[0],) + z.shape[1:], z.dtype) for z in zero_outs
    ]
    args = [jax.device_put(a) for a in concat_in + concat_zero]
    return fn, out_names, out_avals, args


def kernel(input_ids, token_emb, pos_emb, layers, out_w, out_b):
    in_maps = prep_inputs(input_ids, token_emb, pos_emb, layers, out_w, out_b)
    fn, out_names, out_avals, args = _device_args(in_maps)
    outs = fn(*args)
    out = np.asarray(outs[0])                      # [8*1024, 4000]
    out = out.reshape(N_CORES, T, VS)
    full = np.concatenate([out[c] for c in range(N_CORES)], axis=1)  # [1024, 32000]
    return full.reshape(B, S, VOCAB).astype(np.float32)


# revision 19
# speedup vs baseline: 1.0301x; 1.0301x over previous
"""Photonic transformer on 8 Trainium2 NeuronCores.

Sharding: tensor-parallel attention (1 head per core, all 16 wavelengths
local), token-parallel FFN (two 64-token half-shards per core), vocab-parallel
output projection (4000 cols per core). Comm per layer: per-batch
ReduceScatter of (out-proj partials + x/8) and per-batch AllGather of the
updated residual half — batch 0's RS/FFN/AG overlaps batch 1's attention, and
the per-batch-half LN1/QKV lets layer l+1's batch-0 attention overlap layer
l's tail (cross-layer pipeline). The wavelength softmax uses one ACT exp per
wavelength (per-partition scale = cos^2(phase)/sqrt(64), free-axis accum_out
for row sums) and one fused scalar_tensor_tensor accumulate per wavelength
(split DVE/GPSIMD); the summed attention matrix hits the AV matmul once. The
Kerr factor cos(2.7e-12*s^2) == 1.0 in fp32 and is dropped. LayerNorm affine
params are folded into the following weights on the host.
"""
import os
import numpy as np
import ml_dtypes

N_CORES = 8
B, S, D, H, HEAD, W, L, VOCAB = 2, 512, 512, 8, 64, 16, 6, 32000
T = B * S              # 1024 tokens
NT = T // 128          # 8 token tiles
KT = D // 128          # 4 k-tiles over D
FF = 4 * D             # 2048
KF = FF // 128         # 16 k-tiles over FF
VS = VOCAB // N_CORES  # 4000 vocab cols per core
NCHV = 8
VCH = VS // NCHV       # 500
LN_EPS = 1e-5

# wavelength indices whose accumulate runs on gpsimd (empirical knob)
GPS_W = ()
USE_TREE = os.environ.get("PK_TREE", "1") == "1"
L_RUN = int(os.environ.get("PK_LAYERS", L))
USE_COLL = os.environ.get("PK_COLL", "1") == "1"
DO_VOCAB = os.environ.get("PK_VOCAB", "1") == "1"

_CACHE = {}


def _build_nc():
    from concourse import bacc, mybir, tile
    from concourse.bass import IndirectOffsetOnAxis

    F32, BF16, I32 = mybir.dt.float32, mybir.dt.bfloat16, mybir.dt.int32
    AF, ALU = mybir.ActivationFunctionType, mybir.AluOpType

    nc = bacc.Bacc(None, num_devices=N_CORES)

    # ---------------- I/O ----------------
    ids_d = nc.dram_tensor("ids", [128, NT], I32, kind="ExternalInput")
    temb_d = nc.dram_tensor("temb", [VOCAB // N_CORES + 1, D], BF16, kind="ExternalInput")
    pos_d = nc.dram_tensor("pos", [S, D], F32, kind="ExternalInput")
    ident_d = nc.dram_tensor("ident", [128, 128], BF16, kind="ExternalInput")
    lw = []
    for l in range(L):
        lw.append(dict(
            qkvw=nc.dram_tensor(f"qkvw{l}", [KT, 128, 3 * HEAD], BF16, kind="ExternalInput"),
            qkvb=nc.dram_tensor(f"qkvb{l}", [1, 3 * HEAD], BF16, kind="ExternalInput"),
            ow=nc.dram_tensor(f"ow{l}", [HEAD, D], BF16, kind="ExternalInput"),
            ob8=nc.dram_tensor(f"ob8{l}", [1, D], BF16, kind="ExternalInput"),
            c2s=nc.dram_tensor(f"c2s{l}", [128, W], F32, kind="ExternalInput"),
            iw16=nc.dram_tensor(f"iw16{l}", [128, W], F32, kind="ExternalInput"),
            f1w=nc.dram_tensor(f"f1w{l}", [KT, 128, FF], BF16, kind="ExternalInput"),
            f1b=nc.dram_tensor(f"f1b{l}", [1, FF], BF16, kind="ExternalInput"),
            f2w=nc.dram_tensor(f"f2w{l}", [KF, 128, D], BF16, kind="ExternalInput"),
            f2b=nc.dram_tensor(f"f2b{l}", [1, D], BF16, kind="ExternalInput"),
        ))
    outw_d = nc.dram_tensor("outw", [KT, 128, VS], BF16, kind="ExternalInput")
    outb_d = nc.dram_tensor("outb", [1, VS], BF16, kind="ExternalInput")
    out_d = nc.dram_tensor("out", [T, VS], F32, kind="ExternalOutput")

    with tile.TileContext(nc) as tc:
        with (
            tc.tile_pool(name="const", bufs=1) as cpool,
            tc.tile_pool(name="wq", bufs=2) as wq,        # per-layer small weights
            tc.tile_pool(name="wf", bufs=1) as wf,        # per-layer FFN weights
            tc.tile_pool(name="xres", bufs=2) as xres,    # residual stream (x)
            tc.tile_pool(name="xp", bufs=1) as xp,        # per-layer persistents
            tc.tile_pool(name="act", bufs=2) as ap,       # activations
            tc.tile_pool(name="ep", bufs=2) as ep,        # exp tiles
            tc.tile_pool(name="sm", bufs=2) as sm,        # small stats
            tc.tile_pool(name="dram", bufs=2, space="DRAM") as dp,
            tc.tile_pool(name="ps_sc", bufs=1, space="PSUM") as ps_sc,
            tc.tile_pool(name="ps_mm", bufs=2, space="PSUM") as ps_mm,
            tc.tile_pool(name="ps_tp", bufs=2, space="PSUM") as ps_tp,
            tc.tile_pool(name="ps_h1", bufs=1, space="PSUM") as ps_h1,
        ):
            ones = cpool.tile([128, T], BF16)
            nc.gpsimd.memset(ones[:], 1.0)
            ident = cpool.tile([128, 128], BF16)
            nc.sync.dma_start(ident[:], ident_d[:])
            ids_sb = cpool.tile([128, NT], I32)
            nc.sync.dma_start(ids_sb[:], ids_d[:])
            pos_sb = ep.tile([128, NT // 2, D], F32, tag="E")
            for t in range(NT // 2):
                nc.sync.dma_start(pos_sb[:, t, :], pos_d[t * 128:(t + 1) * 128, :])

            # ---------------- embedding (vocab-sharded gather + AllReduce) ----
            x_sb = xres.tile([128, NT, D], F32, tag="x")
            gath = ep.tile([128, NT, D], BF16, tag="E")
            nc.gpsimd.memset(gath[:], 0.0)
            for t in range(NT):
                nc.gpsimd.indirect_dma_start(
                    out=gath[:, t, :], out_offset=None,
                    in_=temb_d[:, :],
                    in_offset=IndirectOffsetOnAxis(ap=ids_sb[:, t:t + 1], axis=0),
                    bounds_check=VOCAB // N_CORES - 1, oob_is_err=False,
                )
            emb_in = dp.tile([T, D], BF16, tag="emb_in")
            emb_out = dp.tile([T, D], BF16, tag="emb_out", addr_space="Shared")
            for t in range(NT):
                nc.sync.dma_start(emb_in[t * 128:(t + 1) * 128, :], gath[:, t, :])
            if USE_COLL:
                nc.gpsimd.collective_compute(
                    "AllReduce", ALU.add,
                    replica_groups=[list(range(N_CORES))],
                    ins=[emb_in[:].opt()], outs=[emb_out[:].opt()],
                )
            else:
                nc.sync.dma_start(emb_out[:], emb_in[:])
            for t in range(NT):
                nc.sync.dma_start(gath[:, t, :], emb_out[t * 128:(t + 1) * 128, :])
            for t in range(NT):
                nc.vector.tensor_tensor(x_sb[:, t, :], gath[:, t, :],
                                        pos_sb[:, t % (NT // 2), :], ALU.add)

            def quake_rstd(veps, n, rows=128):
                """veps [rows, n] f32 AP (var+eps) -> rstd tile, quake + 3 Newton."""
                y0i_t = sm.tile([128, n], I32, tag="qk_i", name="qk_i")
                y0i = y0i_t[0:rows]
                nc.vector.tensor_scalar(y0i, veps.bitcast(I32), 1, None,
                                        op0=ALU.logical_shift_right)
                nc.vector.tensor_scalar(y0i, y0i, 0x5F3759DF, -1,
                                        op0=ALU.subtract, op1=ALU.mult)
                yt = sm.tile([128, n], F32, tag="qk_y", name="qk_y")
                y = yt[0:rows]
                nc.vector.tensor_copy(y, y0i.bitcast(F32))
                t1_t = sm.tile([128, n], F32, tag="qk_t", name="qk_t")
                t1 = t1_t[0:rows]
                for _ in range(3):
                    nc.vector.tensor_tensor(t1, y, y, ALU.mult)
                    nc.vector.tensor_tensor(t1, t1, veps, ALU.mult)
                    nc.vector.tensor_scalar(t1, t1, -0.5, 1.5, op0=ALU.mult, op1=ALU.add)
                    nc.vector.tensor_tensor(y, y, t1, ALU.mult)
                return yt

            # ================ layers ================
            for l in range(L_RUN):
                w = lw[l]
                qkvw = wq.tile([128, KT, 3 * HEAD], BF16, tag="qkvw")
                qkvb = wq.tile([128, 3 * HEAD], BF16, tag="qkvb")
                ow_sb = wq.tile([128, D], BF16, tag="ow")
                ob8 = wq.tile([128, D], BF16, tag="ob8")
                c2s = wq.tile([128, W], F32, tag="c2s")
                iw16 = wq.tile([128, W], F32, tag="iw16")
                nc.sync.dma_start(qkvw[:], w["qkvw"][:].rearrange("k p c -> p k c"))
                nc.sync.dma_start(qkvb[0:1, :], w["qkvb"][:])
                nc.sync.dma_start(ow_sb[0:HEAD, :], w["ow"][:])
                nc.sync.dma_start(ob8[0:1, :], w["ob8"][:])
                nc.sync.dma_start(c2s[:], w["c2s"][:])
                nc.sync.dma_start(iw16[:], w["iw16"][:])
                f1w = wf.tile([128, KT, FF], BF16, tag="f1w")
                f1b = wf.tile([128, FF], BF16, tag="f1b")
                f2w = wf.tile([128, KF, D], BF16, tag="f2w")
                f2b = wf.tile([128, D], BF16, tag="f2b")
                nc.sync.dma_start(f1w[:], w["f1w"][:].rearrange("k p c -> p k c"))
                nc.sync.dma_start(f1b[0:1, :], w["f1b"][:])
                nc.sync.dma_start(f2w[:], w["f2w"][:].rearrange("k p c -> p k c"))
                nc.sync.dma_start(f2b[0:1, :], w["f2b"][:])

                zT = xp.tile([128, KT, T], BF16, tag="zT")
                qT = ap.tile([128, T], BF16, tag="qT")
                kTt = ap.tile([128, T], BF16, tag="kTt")
                v_sb = xp.tile([128, NT, HEAD], BF16, tag="v_sb")
                OT_sb = xp.tile([128, NT, 128], BF16, tag="OT")
                x_next = xres.tile([128, NT, D], F32, tag="x", name="x_next")

                def ffn_half(rs_out_b):
                    """LN2 + photonic FFN on this core's 64-token half."""
                    x1 = ap.tile([128, D], F32, tag="x1h", name="x1h")
                    nc.sync.dma_start(x1[0:64, :], rs_out_b[:])
                    st6b = sm.tile([128, 1, 6], F32, tag="st6b", name="st6b")
                    mvb = sm.tile([128, 1, 2], F32, tag="mvb", name="mvb")
                    nc.vector.bn_stats(st6b[0:64, 0, :], x1[0:64, :])
                    nc.vector.bn_aggr(mvb[0:64, 0, :], st6b[0:64, 0, :])
                    negmu2 = sm.tile([128, 1], F32, tag="negmu2", name="negmu2")
                    veps2 = sm.tile([128, 1], F32, tag="veps2", name="veps2")
                    nc.vector.tensor_scalar_mul(negmu2[0:64], mvb[0:64, :, 0], -1.0)
                    nc.vector.tensor_scalar_add(veps2[0:64], mvb[0:64, :, 1], LN_EPS)
                    rstd2 = quake_rstd(veps2[0:64], 1, rows=64)
                    z2 = ap.tile([128, D], BF16, tag="z2", name="z2")
                    nc.vector.tensor_scalar(z2[0:64, :], x1[0:64, :],
                                            negmu2[0:64, 0:1], rstd2[0:64, 0:1],
                                            op0=ALU.add, op1=ALU.mult)
                    z2T = ap.tile([128, KT, 64], BF16, tag="z2T", name="z2T")
                    tp = ps_tp.tile([128, KT, 128], BF16, tag="tp", name="tp")
                    for k in range(KT):
                        nc.tensor.matmul(tp[:, k, 0:64], z2[0:64, k * 128:(k + 1) * 128],
                                         ident[0:64, 0:64], is_transpose=True,
                                         start=(k == 0), stop=(k == KT - 1))
                    nc.vector.tensor_copy(z2T[:], tp[:, :, 0:64])
                    actF = ap.tile([128, KF, 64], BF16, tag="actF", name="actF")
                    for hf in range(2):
                        h1 = ps_h1.tile([128, 2, 512], F32, tag="h1", name="h1")
                        for nch in range(2):
                            ncol = hf * 1024 + nch * 512
                            for k in range(KT):
                                nc.tensor.matmul(h1[0:64, nch, :], z2T[:, k, :],
                                                 f1w[:, k, ncol:ncol + 512],
                                                 start=(k == 0), stop=False)
                            nc.tensor.matmul(h1[0:64, nch, :], ones[0:1, 0:64],
                                             f1b[0:1, ncol:ncol + 512], start=False, stop=True)
                        for nch in range(2):
                            tnh = ap.tile([128, 512], BF16, tag="tnh", name="tnh")
                            nc.scalar.activation(tnh[0:64, :], h1[0:64, nch, :], AF.Tanh,
                                                 scale=2.0)
                            den = ap.tile([128, 512], F32, tag="den", name="den")
                            nc.scalar.activation(den[0:64, :], h1[0:64, nch, :], AF.Abs)
                            nc.vector.tensor_scalar_add(den[0:64, :], den[0:64, :], 1.0)
                            nc.vector.reciprocal_approx_fast(den[0:64, :], den[0:64, :])
                            af = ap.tile([128, 512], BF16, tag="af", name="af")
                            nc.vector.scalar_tensor_tensor(af[0:64, :], den[0:64, :], 1.0,
                                                           tnh[0:64, :],
                                                           op0=ALU.mult, op1=ALU.mult)
                            tp = ps_tp.tile([128, KT, 128], BF16, tag="tp", name="tp")
                            for j in range(4):
                                nc.tensor.matmul(tp[:, j, 0:64],
                                                 af[0:64, j * 128:(j + 1) * 128],
                                                 ident[0:64, 0:64], is_transpose=True,
                                                 start=(j == 0), stop=(j == 3))
                            nc.vector.tensor_copy(
                                actF[:, hf * 8 + nch * 4:hf * 8 + nch * 4 + 4, :],
                                tp[:, :, 0:64])
                    x2 = ps_mm.tile([128, D], F32, tag="mm", name="x2")
                    for kk in range(KF):
                        nc.tensor.matmul(x2[0:64, :], actF[:, kk, :], f2w[:, kk, :],
                                         start=(kk == 0), stop=False)
                    nc.tensor.matmul(x2[0:64, :], ones[0:1, 0:64], f2b[0:1, :],
                                     start=False, stop=True)
                    y_sb = ap.tile([128, D], F32, tag="y_sb", name="y_sb")
                    nc.vector.scalar_tensor_tensor(y_sb[0:64, :], x1[0:64, :], 1.0,
                                                   x2[0:64, :], op0=ALU.mult, op1=ALU.add)
                    ag_in = dp.tile([64, D], F32, tag="ag_in", name="ag_in")
                    nc.sync.dma_start(ag_in[:], y_sb[0:64, :])
                    return ag_in

                for b in range(B):
                    bsl = slice(b * 512, (b + 1) * 512)
                    # ---- LN1 + z^T + QKV for this batch half
                    st6 = sm.tile([128, 4, 6], F32, tag="st6", name="st6")
                    mv = sm.tile([128, 4, 2], F32, tag="mv", name="mv")
                    for q in range(4):
                        t = b * 4 + q
                        nc.vector.bn_stats(st6[:, q, :], x_sb[:, t, :])
                        nc.vector.bn_aggr(mv[:, q, :], st6[:, q, :])
                    negmu = sm.tile([128, 4], F32, tag="negmu", name="negmu")
                    veps = sm.tile([128, 4], F32, tag="veps", name="veps")
                    nc.vector.tensor_scalar_mul(negmu[:], mv[:, :, 0], -1.0)
                    nc.vector.tensor_scalar_add(veps[:], mv[:, :, 1], LN_EPS)
                    rstd = quake_rstd(veps[:], 4)
                    for q in range(4):
                        t = b * 4 + q
                        z = ap.tile([128, D], BF16, tag="z", name="z")
                        nc.vector.tensor_scalar(z[:], x_sb[:, t, :],
                                                negmu[:, q:q + 1], rstd[:, q:q + 1],
                                                op0=ALU.add, op1=ALU.mult)
                        tp = ps_tp.tile([128, KT, 128], BF16, tag="tp", name="tp")
                        for k in range(KT):
                            nc.tensor.matmul(tp[:, k, :], z[:, k * 128:(k + 1) * 128],
                                             ident[:], is_transpose=True,
                                             start=(k == 0), stop=(k == KT - 1))
                        nc.vector.tensor_copy(zT[:, :, t * 128:(t + 1) * 128], tp[:])
                    for which, dst in ((0, qT), (1, kTt)):
                        ps_full = ps_mm.tile([128, 512], F32, tag="mm", name="qk_ps")
                        ps = ps_full[0:HEAD, :]
                        cs = slice(which * HEAD, (which + 1) * HEAD)
                        for k in range(KT):
                            nc.tensor.matmul(ps, qkvw[:, k, cs], zT[:, k, bsl],
                                             start=(k == 0), stop=False)
                        nc.tensor.matmul(ps, qkvb[0:1, cs], ones[0:1, bsl],
                                         start=False, stop=True)
                        nc.vector.tensor_copy(dst[0:HEAD, bsl], ps)
                    for q in range(4):
                        t = b * 4 + q
                        ps = ps_mm.tile([128, HEAD], F32, tag="mm", name="v_ps")
                        for k in range(KT):
                            nc.tensor.matmul(ps[:], zT[:, k, t * 128:(t + 1) * 128],
                                             qkvw[:, k, 2 * HEAD:3 * HEAD],
                                             start=(k == 0), stop=False)
                        nc.tensor.matmul(ps[:], ones[0:1, :128],
                                         qkvb[0:1, 2 * HEAD:3 * HEAD], start=False, stop=True)
                        nc.vector.tensor_copy(v_sb[:, t, :], ps[:])

                    # ---- attention over the batch
                    rs_in_b = dp.tile([S, D], F32, tag="rs_in", name="rs_in")
                    AT = ap.tile([128, KT, 512], BF16, tag="AT", name="AT")
                    ot_b = ps_h1.tile([128, 512], F32, tag="otb", name="ot_b")
                    for st in range(4):
                        sc = ps_sc.tile([128, 512], F32, tag="sc", name="sc")
                        nc.tensor.matmul(
                            sc[:], qT[0:HEAD, b * 512 + st * 128: b * 512 + (st + 1) * 128],
                            kTt[0:HEAD, bsl])
                        E = ep.tile([128, W, 512], BF16, tag="E", name="E")
                        Z = sm.tile([128, W], F32, tag="Z", name="Z")
                        for wl in range(W):
                            nc.scalar.activation(E[:, wl, :], sc[:], AF.Exp,
                                                 scale=c2s[:, wl:wl + 1],
                                                 accum_out=Z[:, wl:wl + 1])
                        R = sm.tile([128, W], F32, tag="R", name="R")
                        nc.vector.reciprocal_approx_fast(R[:], Z[:])
                        nc.vector.tensor_tensor(R[:], R[:], iw16[:], ALU.mult)
                        A = ap.tile([128, 512], BF16, tag="A", name="A")
                        if USE_TREE:
                            # P_w = E_w * r_w in place, then pairwise tree adds
                            for wl in range(W):
                                nc.vector.tensor_scalar_mul(E[:, wl, :], E[:, wl, :],
                                                            R[:, wl:wl + 1])
                            nc.vector.tensor_tensor(E[:, 0:8, :], E[:, 0:8, :],
                                                    E[:, 8:16, :], ALU.add)
                            nc.vector.tensor_tensor(E[:, 0:4, :], E[:, 0:4, :],
                                                    E[:, 4:8, :], ALU.add)
                            nc.vector.tensor_tensor(E[:, 0:2, :], E[:, 0:2, :],
                                                    E[:, 2:4, :], ALU.add)
                            nc.vector.tensor_tensor(A[:], E[:, 0, :], E[:, 1, :], ALU.add)
                        else:
                            nc.vector.tensor_scalar_mul(A[:], E[:, 0, :], R[:, 0:1])
                            for wl in range(1, W):
                                nc.vector.scalar_tensor_tensor(
                                    A[:], E[:, wl, :], R[:, wl:wl + 1], A[:],
                                    op0=ALU.mult, op1=ALU.add)
                        tp = ps_tp.tile([128, KT, 128], BF16, tag="tp", name="tp")
                        for j in range(KT):
                            nc.tensor.matmul(tp[:, j, :], A[:, j * 128:(j + 1) * 128],
                                             ident[:], is_transpose=True,
                                             start=(j == 0), stop=(j == KT - 1))
                        nc.vector.tensor_copy(AT[:, :, st * 128:(st + 1) * 128], tp[:])
                        for j in range(KT):
                            nc.tensor.matmul(ot_b[0:HEAD, st * 128:(st + 1) * 128],
                                             v_sb[:, b * 4 + j, :],
                                             AT[:, j, st * 128:(st + 1) * 128],
                                             start=(st == 0 and j == 0),
                                             stop=(st == 3 and j == KT - 1))
                    nc.vector.tensor_copy(OT_sb[0:HEAD, b * 4:(b + 1) * 4, :],
                                          ot_b[0:HEAD, :].rearrange("p (s c) -> p s c", c=128))
                    for st in range(4):
                        tok = b * 4 + st
                        pp = ps_mm.tile([128, D], F32, tag="mm", name="pp")
                        nc.tensor.matmul(pp[:], OT_sb[0:HEAD, tok, :], ow_sb[0:HEAD, :],
                                         start=True, stop=False)
                        nc.tensor.matmul(pp[:], ones[0:1, :128], ob8[0:1, :],
                                         start=False, stop=True)
                        p_sb = ap.tile([128, D], F32, tag="p_sb", name="p_sb")
                        nc.vector.scalar_tensor_tensor(p_sb[:], x_sb[:, tok, :], 0.125,
                                                       pp[:], op0=ALU.mult, op1=ALU.add)
                        nc.sync.dma_start(rs_in_b[st * 128:(st + 1) * 128, :], p_sb[:])

                    # ---- per-batch ReduceScatter -> half FFN -> per-batch AllGather
                    rs_out_b = dp.tile([64, D], F32, tag="rs_out", name="rs_out")
                    if USE_COLL:
                        nc.gpsimd.collective_compute(
                            "ReduceScatter", ALU.add,
                            replica_groups=[list(range(N_CORES))],
                            ins=[rs_in_b[:].opt()], outs=[rs_out_b[:].opt()],
                        )
                    else:
                        nc.sync.dma_start(rs_out_b[:], rs_in_b[0:64, :])
                    ag_in_b = ffn_half(rs_out_b)
                    ag_out_b = dp.tile([S, D], F32, tag="ag_out", name="ag_out",
                                       addr_space="Shared")
                    if USE_COLL:
                        nc.gpsimd.collective_compute(
                            "AllGather", ALU.bypass,
                            replica_groups=[list(range(N_CORES))],
                            ins=[ag_in_b[:].opt()], outs=[ag_out_b[:].opt()],
                        )
                    else:
                        for _r in range(N_CORES):
                            nc.sync.dma_start(ag_out_b[_r * 64:(_r + 1) * 64, :], ag_in_b[:])
                    # next-layer residual stream, this batch's 4 token tiles
                    nc.sync.dma_start(
                        x_next[:, b * 4:(b + 1) * 4, :],
                        ag_out_b[:].rearrange("(q p2 o) d -> (p2 o) q d", q=4, p2=2, o=64))

                x_sb = x_next

            # ================ vocab projection ================
            if not DO_VOCAB:
                fin = ap.tile([128, 8], F32, tag="fin")
                nc.vector.tensor_copy(fin[:], x_sb[:, 0, 0:8])
                nc.sync.dma_start(out_d[0:128, 0:8], fin[:])
            xvT = xp.tile([128, KT, T], BF16, tag="zT", name="xvT")
            for t in (range(NT) if DO_VOCAB else []):
                xc = ap.tile([128, D], BF16, tag="z", name="xc")
                nc.vector.tensor_copy(xc[:], x_sb[:, t, :])
                tp = ps_tp.tile([128, KT, 128], BF16, tag="tp", name="tp")
                for k in range(KT):
                    nc.tensor.matmul(tp[:, k, :], xc[:, k * 128:(k + 1) * 128], ident[:],
                                     is_transpose=True, start=(k == 0), stop=(k == KT - 1))
                nc.vector.tensor_copy(xvT[:, :, t * 128:(t + 1) * 128], tp[:])
            for nch in (range(NCHV) if DO_VOCAB else []):
                ow_c = wq.tile([128, KT, VCH], BF16, tag="outwc", name="ow_c")
                nc.sync.dma_start(
                    ow_c[:], outw_d[:, :, nch * VCH:(nch + 1) * VCH].rearrange("k p c -> p k c"))
                ob_c = wq.tile([128, VCH], BF16, tag="outbc", name="ob_c")
                nc.sync.dma_start(ob_c[0:1, :], outb_d[:, nch * VCH:(nch + 1) * VCH])
                for m in range(NT):
                    vp = ps_mm.tile([128, VCH], F32, tag="mm", name="vp")
                    for k in range(KT):
                        nc.tensor.matmul(vp[:], xvT[:, k, m * 128:(m + 1) * 128],
                                         ow_c[:, k, :],
                                         start=(k == 0), stop=False)
                    nc.tensor.matmul(vp[:], ones[0:1, :128], ob_c[0:1, :],
                                     start=False, stop=True)
                    ob = ap.tile([128, VCH], F32, tag="ob", name="ob")
                    if (m * NCHV + nch) % 2 == 0:
                        nc.vector.tensor_copy(ob[:], vp[:])
                    else:
                        nc.scalar.copy(ob[:], vp[:])
                    nc.sync.dma_start(out_d[m * 128:(m + 1) * 128, nch * VCH:(nch + 1) * VCH],
                                      ob[:])

    nc.compile()
    return nc


def prep_inputs(input_ids, token_emb, pos_emb, layers, out_w, out_b):
    """Host-side input marshalling: per-core weight slices, LN folding, layout."""
    bf16 = ml_dtypes.bfloat16
    f32 = np.float32
    ids = np.asarray(input_ids).reshape(-1).astype(np.int64)       # [1024]
    temb = np.asarray(token_emb, f32)
    pos = np.asarray(pos_emb, f32)[:S]
    ident = np.eye(128, dtype=bf16)
    out_w = np.asarray(out_w, f32)
    out_b = np.asarray(out_b, f32)

    vsh = VOCAB // N_CORES
    in_maps = []
    for r in range(N_CORES):
        idl = ids - r * vsh
        idl = np.where((idl >= 0) & (idl < vsh), idl, vsh).astype(np.int32)
        ids_pt = idl.reshape(NT, 128).T.copy()                      # [128, NT]
        temb_sh = np.concatenate([temb[r * vsh:(r + 1) * vsh],
                                  np.zeros((1, D), f32)], axis=0).astype(bf16)
        m = dict(ids=ids_pt, temb=temb_sh, pos=pos, ident=ident)
        for l, p in enumerate(layers):
            qw = np.asarray(p["qw"], f32) * np.asarray(p["ln1g"], f32)[:, None]
            kw = np.asarray(p["kw"], f32) * np.asarray(p["ln1g"], f32)[:, None]
            vw = np.asarray(p["vw"], f32) * np.asarray(p["ln1g"], f32)[:, None]
            qb = np.asarray(p["qb"], f32) + np.asarray(p["ln1b"], f32) @ np.asarray(p["qw"], f32)
            kb = np.asarray(p["kb"], f32) + np.asarray(p["ln1b"], f32) @ np.asarray(p["kw"], f32)
            vb = np.asarray(p["vb"], f32) + np.asarray(p["ln1b"], f32) @ np.asarray(p["vw"], f32)
            hsl = slice(r * HEAD, (r + 1) * HEAD)
            qkvw = np.concatenate([qw[:, hsl], kw[:, hsl], vw[:, hsl]], axis=1)  # [512,192]
            m[f"qkvw{l}"] = qkvw.astype(bf16).reshape(KT, 128, 3 * HEAD)
            m[f"qkvb{l}"] = np.concatenate([qb[hsl], kb[hsl], vb[hsl]])[None, :].astype(bf16)
            m[f"ow{l}"] = np.asarray(p["ow"], f32)[hsl, :].astype(bf16)
            m[f"ob8{l}"] = (np.asarray(p["ob"], f32) / N_CORES)[None, :].astype(bf16)
            c2 = (np.cos(np.asarray(p["phase"], f32)[:, r]) ** 2) * np.float32(1.0 / np.sqrt(HEAD))
            m[f"c2s{l}"] = np.tile(c2[None, :], (128, 1)).astype(f32)
            iw = np.asarray(p["iw"], f32) / np.float32(W)
            m[f"iw16{l}"] = np.tile(iw[None, :], (128, 1)).astype(f32)
            f1w = np.asarray(p["f1w"], f32) * np.asarray(p["ln2g"], f32)[:, None]
            f1b = np.asarray(p["f1b"], f32) + np.asarray(p["ln2b"], f32) @ np.asarray(p["f1w"], f32)
            m[f"f1w{l}"] = f1w.astype(bf16).reshape(KT, 128, FF)
            m[f"f1b{l}"] = f1b[None, :].astype(bf16)
            m[f"f2w{l}"] = np.asarray(p["f2w"], f32).astype(bf16).reshape(KF, 128, D)
            m[f"f2b{l}"] = np.asarray(p["f2b"], f32)[None, :].astype(bf16)
        vsl = slice(r * VS, (r + 1) * VS)
        m["outw"] = out_w[:, vsl].astype(bf16).reshape(KT, 128, VS)
        m["outb"] = out_b[vsl][None, :].astype(bf16)
        in_maps.append(m)
    return in_maps


def _get_exec():
    if "exec" in _CACHE:
        return _CACHE["exec"]
    import jax
    from jax.sharding import Mesh, PartitionSpec
    from jax.experimental.shard_map import shard_map
    from concourse import bass2jax, mybir

    nc = _build_nc()
    bass2jax.install_neuronx_cc_hook()
    partition_name = nc.partition_id_tensor.name if nc.partition_id_tensor else None
    in_names, out_names, out_avals, zero_outs = [], [], [], []
    for alloc in nc.m.functions[0].allocations:
        if not isinstance(alloc, mybir.MemoryLocationSet):
            continue
        name = alloc.memorylocations[0].name
        if alloc.kind == "ExternalInput":
            if name != partition_name:
                in_names.append(name)
        elif alloc.kind == "ExternalOutput":
            out_names.append(name)
            shape = tuple(alloc.tensor_shape)
            dt = mybir.dt.np(alloc.dtype)
            out_avals.append(jax.core.ShapedArray(shape, dt))
            zero_outs.append(np.zeros(shape, dt))
    n_params = len(in_names)
    all_names = list(in_names) + list(out_names) + ([partition_name] if partition_name else [])

    def _body(*args):
        operands = list(args)
        if partition_name is not None:
            operands.append(bass2jax.partition_id_tensor())
        outs = bass2jax._bass_exec_p.bind(
            *operands,
            out_avals=tuple(out_avals),
            in_names=tuple(all_names),
            out_names=tuple(out_names),
            lowering_input_output_aliases=(),
            sim_require_finite=True,
            sim_require_nnan=True,
            nc=nc,
        )
        return tuple(outs)

    devices = jax.devices()[:N_CORES]
    mesh = Mesh(np.asarray(devices), ("core",))
    in_specs = (PartitionSpec("core"),) * (n_params + len(out_names))
    out_specs = (PartitionSpec("core"),) * len(out_names)
    fn = jax.jit(
        shard_map(_body, mesh=mesh, in_specs=in_specs, out_specs=out_specs,
                  check_rep=False),
        keep_unused=True,
    )
    _CACHE["exec"] = (fn, in_names, out_names, out_avals, zero_outs)
    return _CACHE["exec"]


def _device_args(in_maps):
    import jax
    fn, in_names, out_names, out_avals, zero_outs = _get_exec()
    concat_in = [
        np.concatenate([np.asarray(in_maps[c][n]) for c in range(N_CORES)], axis=0)
        for n in in_names
    ]
    concat_zero = [
        np.zeros((N_CORES * z.shape[0],) + z.shape[1:], z.dtype) for z in zero_outs
    ]
    args = [jax.device_put(a) for a in concat_in + concat_zero]
    return fn, out_names, out_avals, args


def kernel(input_ids, token_emb, pos_emb, layers, out_w, out_b):
    in_maps = prep_inputs(input_ids, token_emb, pos_emb, layers, out_w, out_b)
    fn, out_names, out_avals, args = _device_args(in_maps)
    outs = fn(*args)
    out = np.asarray(outs[0])                      # [8*1024, 4000]
    out = out.reshape(N_CORES, T, VS)
    full = np.concatenate([out[c] for c in range(N_CORES)], axis=1)  # [1024, 32000]
    return full.reshape(B, S, VOCAB).astype(np.float32)


# revision 20
# speedup vs baseline: 77.2133x; 74.9600x over previous
"""Photonic transformer on 8 Trainium2 NeuronCores.

Sharding: tensor-parallel attention (1 head per core, all 16 wavelengths
local), token-parallel FFN (two 64-token half-shards per core), vocab-parallel
output projection (4000 cols per core). Comm per layer: per-batch
ReduceScatter of (out-proj partials + x/8) and per-batch AllGather of the
updated residual half — batch 0's RS/FFN/AG overlaps batch 1's attention, and
the per-batch-half LN1/QKV lets layer l+1's batch-0 attention overlap layer
l's tail (cross-layer pipeline). The wavelength softmax uses one ACT exp per
wavelength (per-partition scale = cos^2(phase)/sqrt(64), free-axis accum_out
for row sums), a DVE scale per wavelength and a pairwise tree-add reduction;
the summed attention matrix hits the AV matmul once. The
Kerr factor cos(2.7e-12*s^2) == 1.0 in fp32 and is dropped. LayerNorm affine
params are folded into the following weights on the host.
"""
import os
import numpy as np
import ml_dtypes

N_CORES = 8
B, S, D, H, HEAD, W, L, VOCAB = 2, 512, 512, 8, 64, 16, 6, 32000
T = B * S              # 1024 tokens
NT = T // 128          # 8 token tiles
KT = D // 128          # 4 k-tiles over D
FF = 4 * D             # 2048
KF = FF // 128         # 16 k-tiles over FF
VS = VOCAB // N_CORES  # 4000 vocab cols per core
NCHV = 8
VCH = VS // NCHV       # 500
LN_EPS = 1e-5

# wavelength indices whose accumulate runs on gpsimd (empirical knob)
GPS_W = ()
USE_TREE = os.environ.get("PK_TREE", "1") == "1"
L_RUN = int(os.environ.get("PK_LAYERS", L))
USE_COLL = os.environ.get("PK_COLL", "1") == "1"
DO_VOCAB = os.environ.get("PK_VOCAB", "1") == "1"

_CACHE = {}


def _build_nc():
    from concourse import bacc, mybir, tile
    from concourse.bass import IndirectOffsetOnAxis

    F32, BF16, I32 = mybir.dt.float32, mybir.dt.bfloat16, mybir.dt.int32
    AF, ALU = mybir.ActivationFunctionType, mybir.AluOpType

    nc = bacc.Bacc(None, num_devices=N_CORES)

    # ---------------- I/O ----------------
    ids_d = nc.dram_tensor("ids", [128, NT], I32, kind="ExternalInput")
    temb_d = nc.dram_tensor("temb", [VOCAB // N_CORES + 1, D], BF16, kind="ExternalInput")
    pos_d = nc.dram_tensor("pos", [S, D], F32, kind="ExternalInput")
    ident_d = nc.dram_tensor("ident", [128, 128], BF16, kind="ExternalInput")
    lw = []
    for l in range(L):
        lw.append(dict(
            qkvw=nc.dram_tensor(f"qkvw{l}", [KT, 128, 3 * HEAD], BF16, kind="ExternalInput"),
            qkvb=nc.dram_tensor(f"qkvb{l}", [1, 3 * HEAD], BF16, kind="ExternalInput"),
            ow=nc.dram_tensor(f"ow{l}", [HEAD, D], BF16, kind="ExternalInput"),
            ob8=nc.dram_tensor(f"ob8{l}", [1, D], BF16, kind="ExternalInput"),
            c2s=nc.dram_tensor(f"c2s{l}", [128, W], F32, kind="ExternalInput"),
            iw16=nc.dram_tensor(f"iw16{l}", [128, W], F32, kind="ExternalInput"),
            f1w=nc.dram_tensor(f"f1w{l}", [KT, 128, FF], BF16, kind="ExternalInput"),
            f1b=nc.dram_tensor(f"f1b{l}", [1, FF], BF16, kind="ExternalInput"),
            f2w=nc.dram_tensor(f"f2w{l}", [KF, 128, D], BF16, kind="ExternalInput"),
            f2b=nc.dram_tensor(f"f2b{l}", [1, D], BF16, kind="ExternalInput"),
        ))
    outw_d = nc.dram_tensor("outw", [KT, 128, VS], BF16, kind="ExternalInput")
    outb_d = nc.dram_tensor("outb", [1, VS], BF16, kind="ExternalInput")
    out_d = nc.dram_tensor("out", [T, VS], F32, kind="ExternalOutput")

    with tile.TileContext(nc) as tc:
        with (
            tc.tile_pool(name="const", bufs=1) as cpool,
            tc.tile_pool(name="wq", bufs=2) as wq,        # per-layer small weights
            tc.tile_pool(name="wf", bufs=1) as wf,        # per-layer FFN weights
            tc.tile_pool(name="xres", bufs=2) as xres,    # residual stream (x)
            tc.tile_pool(name="xp", bufs=1) as xp,        # per-layer persistents
            tc.tile_pool(name="act", bufs=2) as ap,       # activations
            tc.tile_pool(name="ep", bufs=2) as ep,        # exp tiles
            tc.tile_pool(name="sm", bufs=2) as sm,        # small stats
            tc.tile_pool(name="dram", bufs=2, space="DRAM") as dp,
            tc.tile_pool(name="ps_sc", bufs=1, space="PSUM") as ps_sc,
            tc.tile_pool(name="ps_mm", bufs=2, space="PSUM") as ps_mm,
            tc.tile_pool(name="ps_tp", bufs=2, space="PSUM") as ps_tp,
            tc.tile_pool(name="ps_h1", bufs=1, space="PSUM") as ps_h1,
        ):
            ones = cpool.tile([128, T], BF16)
            nc.gpsimd.memset(ones[:], 1.0)
            ident = cpool.tile([128, 128], BF16)
            nc.sync.dma_start(ident[:], ident_d[:])
            ids_sb = cpool.tile([128, NT], I32)
            nc.sync.dma_start(ids_sb[:], ids_d[:])
            pos_sb = ep.tile([128, NT // 2, D], F32, tag="E")
            for t in range(NT // 2):
                nc.sync.dma_start(pos_sb[:, t, :], pos_d[t * 128:(t + 1) * 128, :])

            # ---------------- embedding (vocab-sharded gather + AllReduce) ----
            x_sb = xres.tile([128, NT, D], F32, tag="x")
            gath = ep.tile([128, NT, D], BF16, tag="E")
            nc.gpsimd.memset(gath[:], 0.0)
            for t in range(NT):
                nc.gpsimd.indirect_dma_start(
                    out=gath[:, t, :], out_offset=None,
                    in_=temb_d[:, :],
                    in_offset=IndirectOffsetOnAxis(ap=ids_sb[:, t:t + 1], axis=0),
                    bounds_check=VOCAB // N_CORES - 1, oob_is_err=False,
                )
            emb_in = dp.tile([T, D], BF16, tag="emb_in")
            emb_out = dp.tile([T, D], BF16, tag="emb_out", addr_space="Shared")
            for t in range(NT):
                nc.sync.dma_start(emb_in[t * 128:(t + 1) * 128, :], gath[:, t, :])
            if USE_COLL:
                nc.gpsimd.collective_compute(
                    "AllReduce", ALU.add,
                    replica_groups=[list(range(N_CORES))],
                    ins=[emb_in[:].opt()], outs=[emb_out[:].opt()],
                )
            else:
                nc.sync.dma_start(emb_out[:], emb_in[:])
            for t in range(NT):
                nc.sync.dma_start(gath[:, t, :], emb_out[t * 128:(t + 1) * 128, :])
            for t in range(NT):
                nc.vector.tensor_tensor(x_sb[:, t, :], gath[:, t, :],
                                        pos_sb[:, t % (NT // 2), :], ALU.add)

            def quake_rstd(veps, n, rows=128):
                """veps [rows, n] f32 AP (var+eps) -> rstd tile, quake + 3 Newton."""
                y0i_t = sm.tile([128, n], I32, tag="qk_i", name="qk_i")
                y0i = y0i_t[0:rows]
                nc.vector.tensor_scalar(y0i, veps.bitcast(I32), 1, None,
                                        op0=ALU.logical_shift_right)
                nc.vector.tensor_scalar(y0i, y0i, 0x5F3759DF, -1,
                                        op0=ALU.subtract, op1=ALU.mult)
                yt = sm.tile([128, n], F32, tag="qk_y", name="qk_y")
                y = yt[0:rows]
                nc.vector.tensor_copy(y, y0i.bitcast(F32))
                t1_t = sm.tile([128, n], F32, tag="qk_t", name="qk_t")
                t1 = t1_t[0:rows]
                for _ in range(3):
                    nc.vector.tensor_tensor(t1, y, y, ALU.mult)
                    nc.vector.tensor_tensor(t1, t1, veps, ALU.mult)
                    nc.vector.tensor_scalar(t1, t1, -0.5, 1.5, op0=ALU.mult, op1=ALU.add)
                    nc.vector.tensor_tensor(y, y, t1, ALU.mult)
                return yt

            # ================ layers ================
            for l in range(L_RUN):
                w = lw[l]
                qkvw = wq.tile([128, KT, 3 * HEAD], BF16, tag="qkvw")
                qkvb = wq.tile([128, 3 * HEAD], BF16, tag="qkvb")
                ow_sb = wq.tile([128, D], BF16, tag="ow")
                ob8 = wq.tile([128, D], BF16, tag="ob8")
                c2s = wq.tile([128, W], F32, tag="c2s")
                iw16 = wq.tile([128, W], F32, tag="iw16")
                nc.sync.dma_start(qkvw[:], w["qkvw"][:].rearrange("k p c -> p k c"))
                nc.sync.dma_start(qkvb[0:1, :], w["qkvb"][:])
                nc.sync.dma_start(ow_sb[0:HEAD, :], w["ow"][:])
                nc.sync.dma_start(ob8[0:1, :], w["ob8"][:])
                nc.sync.dma_start(c2s[:], w["c2s"][:])
                nc.sync.dma_start(iw16[:], w["iw16"][:])
                f1w = wf.tile([128, KT, FF], BF16, tag="f1w")
                f1b = wf.tile([128, FF], BF16, tag="f1b")
                f2w = wf.tile([128, KF, D], BF16, tag="f2w")
                f2b = wf.tile([128, D], BF16, tag="f2b")
                nc.sync.dma_start(f1w[:], w["f1w"][:].rearrange("k p c -> p k c"))
                nc.sync.dma_start(f1b[0:1, :], w["f1b"][:])
                nc.sync.dma_start(f2w[:], w["f2w"][:].rearrange("k p c -> p k c"))
                nc.sync.dma_start(f2b[0:1, :], w["f2b"][:])

                zT = xp.tile([128, KT, T], BF16, tag="zT")
                qT = ap.tile([128, T], BF16, tag="qT")
                kTt = ap.tile([128, T], BF16, tag="kTt")
                v_sb = xp.tile([128, NT, HEAD], BF16, tag="v_sb")
                OT_sb = xp.tile([128, NT, 128], BF16, tag="OT")
                x_next = xres.tile([128, NT, D], F32, tag="x", name="x_next")

                def ffn_half(rs_out_b):
                    """LN2 + photonic FFN on this core's 64-token half."""
                    x1 = ap.tile([128, D], F32, tag="x1h", name="x1h")
                    nc.sync.dma_start(x1[0:64, :], rs_out_b[:])
                    st6b = sm.tile([128, 1, 6], F32, tag="st6b", name="st6b")
                    mvb = sm.tile([128, 1, 2], F32, tag="mvb", name="mvb")
                    nc.vector.bn_stats(st6b[0:64, 0, :], x1[0:64, :])
                    nc.vector.bn_aggr(mvb[0:64, 0, :], st6b[0:64, 0, :])
                    negmu2 = sm.tile([128, 1], F32, tag="negmu2", name="negmu2")
                    veps2 = sm.tile([128, 1], F32, tag="veps2", name="veps2")
                    nc.vector.tensor_scalar_mul(negmu2[0:64], mvb[0:64, :, 0], -1.0)
                    nc.vector.tensor_scalar_add(veps2[0:64], mvb[0:64, :, 1], LN_EPS)
                    rstd2 = quake_rstd(veps2[0:64], 1, rows=64)
                    z2 = ap.tile([128, D], BF16, tag="z2", name="z2")
                    nc.vector.tensor_scalar(z2[0:64, :], x1[0:64, :],
                                            negmu2[0:64, 0:1], rstd2[0:64, 0:1],
                                            op0=ALU.add, op1=ALU.mult)
                    z2T = ap.tile([128, KT, 64], BF16, tag="z2T", name="z2T")
                    tp = ps_tp.tile([128, KT, 128], BF16, tag="tp", name="tp")
                    for k in range(KT):
                        nc.tensor.matmul(tp[:, k, 0:64], z2[0:64, k * 128:(k + 1) * 128],
                                         ident[0:64, 0:64], is_transpose=True,
                                         start=(k == 0), stop=(k == KT - 1))
                    nc.vector.tensor_copy(z2T[:], tp[:, :, 0:64])
                    actF = ap.tile([128, KF, 64], BF16, tag="actF", name="actF")
                    for hf in range(2):
                        h1 = ps_h1.tile([128, 2, 512], F32, tag="h1", name="h1")
                        for nch in range(2):
                            ncol = hf * 1024 + nch * 512
                            for k in range(KT):
                                nc.tensor.matmul(h1[0:64, nch, :], z2T[:, k, :],
                                                 f1w[:, k, ncol:ncol + 512],
                                                 start=(k == 0), stop=False)
                            nc.tensor.matmul(h1[0:64, nch, :], ones[0:1, 0:64],
                                             f1b[0:1, ncol:ncol + 512], start=False, stop=True)
                        for nch in range(2):
                            tnh = ap.tile([128, 512], BF16, tag="tnh", name="tnh")
                            nc.scalar.activation(tnh[0:64, :], h1[0:64, nch, :], AF.Tanh,
                                                 scale=2.0)
                            den = ap.tile([128, 512], F32, tag="den", name="den")
                            nc.scalar.activation(den[0:64, :], h1[0:64, nch, :], AF.Abs)
                            nc.vector.tensor_scalar_add(den[0:64, :], den[0:64, :], 1.0)
                            nc.vector.reciprocal_approx_fast(den[0:64, :], den[0:64, :])
                            af = ap.tile([128, 512], BF16, tag="af", name="af")
                            nc.vector.scalar_tensor_tensor(af[0:64, :], den[0:64, :], 1.0,
                                                           tnh[0:64, :],
                                                           op0=ALU.mult, op1=ALU.mult)
                            tp = ps_tp.tile([128, KT, 128], BF16, tag="tp", name="tp")
                            for j in range(4):
                                nc.tensor.matmul(tp[:, j, 0:64],
                                                 af[0:64, j * 128:(j + 1) * 128],
                                                 ident[0:64, 0:64], is_transpose=True,
                                                 start=(j == 0), stop=(j == 3))
                            nc.vector.tensor_copy(
                                actF[:, hf * 8 + nch * 4:hf * 8 + nch * 4 + 4, :],
                                tp[:, :, 0:64])
                    x2 = ps_mm.tile([128, D], F32, tag="mm", name="x2")
                    for kk in range(KF):
                        nc.tensor.matmul(x2[0:64, :], actF[:, kk, :], f2w[:, kk, :],
                                         start=(kk == 0), stop=False)
                    nc.tensor.matmul(x2[0:64, :], ones[0:1, 0:64], f2b[0:1, :],
                                     start=False, stop=True)
                    y_sb = ap.tile([128, D], F32, tag="y_sb", name="y_sb")
                    nc.vector.scalar_tensor_tensor(y_sb[0:64, :], x1[0:64, :], 1.0,
                                                   x2[0:64, :], op0=ALU.mult, op1=ALU.add)
                    ag_in = dp.tile([64, D], F32, tag="ag_in", name="ag_in")
                    nc.sync.dma_start(ag_in[:], y_sb[0:64, :])
                    return ag_in

                for b in range(B):
                    bsl = slice(b * 512, (b + 1) * 512)
                    # ---- LN1 + z^T + QKV for this batch half
                    st6 = sm.tile([128, 4, 6], F32, tag="st6", name="st6")
                    mv = sm.tile([128, 4, 2], F32, tag="mv", name="mv")
                    for q in range(4):
                        t = b * 4 + q
                        nc.vector.bn_stats(st6[:, q, :], x_sb[:, t, :])
                        nc.vector.bn_aggr(mv[:, q, :], st6[:, q, :])
                    negmu = sm.tile([128, 4], F32, tag="negmu", name="negmu")
                    veps = sm.tile([128, 4], F32, tag="veps", name="veps")
                    nc.vector.tensor_scalar_mul(negmu[:], mv[:, :, 0], -1.0)
                    nc.vector.tensor_scalar_add(veps[:], mv[:, :, 1], LN_EPS)
                    rstd = quake_rstd(veps[:], 4)
                    for q in range(4):
                        t = b * 4 + q
                        z = ap.tile([128, D], BF16, tag="z", name="z")
                        nc.vector.tensor_scalar(z[:], x_sb[:, t, :],
                                                negmu[:, q:q + 1], rstd[:, q:q + 1],
                                                op0=ALU.add, op1=ALU.mult)
                        tp = ps_tp.tile([128, KT, 128], BF16, tag="tp", name="tp")
                        for k in range(KT):
                            nc.tensor.matmul(tp[:, k, :], z[:, k * 128:(k + 1) * 128],
                                             ident[:], is_transpose=True,
                                             start=(k == 0), stop=(k == KT - 1))
                        nc.vector.tensor_copy(zT[:, :, t * 128:(t + 1) * 128], tp[:])
                    for which, dst in ((0, qT), (1, kTt)):
                        ps_full = ps_mm.tile([128, 512], F32, tag="mm", name="qk_ps")
                        ps = ps_full[0:HEAD, :]
                        cs = slice(which * HEAD, (which + 1) * HEAD)
                        for k in range(KT):
                            nc.tensor.matmul(ps, qkvw[:, k, cs], zT[:, k, bsl],
                                             start=(k == 0), stop=False)
                        nc.tensor.matmul(ps, qkvb[0:1, cs], ones[0:1, bsl],
                                         start=False, stop=True)
                        nc.vector.tensor_copy(dst[0:HEAD, bsl], ps)
                    for q in range(4):
                        t = b * 4 + q
                        ps = ps_mm.tile([128, HEAD], F32, tag="mm", name="v_ps")
                        for k in range(KT):
                            nc.tensor.matmul(ps[:], zT[:, k, t * 128:(t + 1) * 128],
                                             qkvw[:, k, 2 * HEAD:3 * HEAD],
                                             start=(k == 0), stop=False)
                        nc.tensor.matmul(ps[:], ones[0:1, :128],
                                         qkvb[0:1, 2 * HEAD:3 * HEAD], start=False, stop=True)
                        nc.vector.tensor_copy(v_sb[:, t, :], ps[:])

                    # ---- attention over the batch
                    rs_in_b = dp.tile([S, D], F32, tag="rs_in", name="rs_in")
                    AT = ap.tile([128, KT, 512], BF16, tag="AT", name="AT")
                    ot_b = ps_h1.tile([128, 512], F32, tag="otb", name="ot_b")
                    for st in range(4):
                        sc = ps_sc.tile([128, 512], F32, tag="sc", name="sc")
                        nc.tensor.matmul(
                            sc[:], qT[0:HEAD, b * 512 + st * 128: b * 512 + (st + 1) * 128],
                            kTt[0:HEAD, bsl])
                        E = ep.tile([128, W, 512], BF16, tag="E", name="E")
                        Z = sm.tile([128, W], F32, tag="Z", name="Z")
                        for wl in range(W):
                            nc.scalar.activation(E[:, wl, :], sc[:], AF.Exp,
                                                 scale=c2s[:, wl:wl + 1],
                                                 accum_out=Z[:, wl:wl + 1])
                        R = sm.tile([128, W], F32, tag="R", name="R")
                        nc.vector.reciprocal_approx_fast(R[:], Z[:])
                        nc.vector.tensor_tensor(R[:], R[:], iw16[:], ALU.mult)
                        A = ap.tile([128, 512], BF16, tag="A", name="A")
                        if USE_TREE:
                            # P_w = E_w * r_w in place, then pairwise tree adds
                            for wl in range(W):
                                nc.vector.tensor_scalar_mul(E[:, wl, :], E[:, wl, :],
                                                            R[:, wl:wl + 1])
                            nc.vector.tensor_tensor(E[:, 0:8, :], E[:, 0:8, :],
                                                    E[:, 8:16, :], ALU.add)
                            nc.vector.tensor_tensor(E[:, 0:4, :], E[:, 0:4, :],
                                                    E[:, 4:8, :], ALU.add)
                            nc.vector.tensor_tensor(E[:, 0:2, :], E[:, 0:2, :],
                                                    E[:, 2:4, :], ALU.add)
                            nc.vector.tensor_tensor(A[:], E[:, 0, :], E[:, 1, :], ALU.add)
                        else:
                            nc.vector.tensor_scalar_mul(A[:], E[:, 0, :], R[:, 0:1])
                            for wl in range(1, W):
                                nc.vector.scalar_tensor_tensor(
                                    A[:], E[:, wl, :], R[:, wl:wl + 1], A[:],
                                    op0=ALU.mult, op1=ALU.add)
                        tp = ps_tp.tile([128, KT, 128], BF16, tag="tp", name="tp")
                        for j in range(KT):
                            nc.tensor.matmul(tp[:, j, :], A[:, j * 128:(j + 1) * 128],
                                             ident[:], is_transpose=True,
                                             start=(j == 0), stop=(j == KT - 1))
                        nc.vector.tensor_copy(AT[:, :, st * 128:(st + 1) * 128], tp[:])
                        for j in range(KT):
                            nc.tensor.matmul(ot_b[0:HEAD, st * 128:(st + 1) * 128],
                                             v_sb[:, b * 4 + j, :],
                                             AT[:, j, st * 128:(st + 1) * 128],
                                             start=(st == 0 and j == 0),
                                             stop=(st == 3 and j == KT - 1))
                    nc.vector.tensor_copy(OT_sb[0:HEAD, b * 4:(b + 1) * 4, :],
                                          ot_b[0:HEAD, :].rearrange("p (s c) -> p s c", c=128))
                    for st in range(4):
                        tok = b * 4 + st
                        pp = ps_mm.tile([128, D], F32, tag="mm", name="pp")
                        nc.tensor.matmul(pp[:], OT_sb[0:HEAD, tok, :], ow_sb[0:HEAD, :],
                                         start=True, stop=False)
                        nc.tensor.matmul(pp[:], ones[0:1, :128], ob8[0:1, :],
                                         start=False, stop=True)
                        p_sb = ap.tile([128, D], F32, tag="p_sb", name="p_sb")
                        nc.vector.scalar_tensor_tensor(p_sb[:], x_sb[:, tok, :], 0.125,
                                                       pp[:], op0=ALU.mult, op1=ALU.add)
                        nc.sync.dma_start(rs_in_b[st * 128:(st + 1) * 128, :], p_sb[:])

                    # ---- per-batch ReduceScatter -> half FFN -> per-batch AllGather
                    rs_out_b = dp.tile([64, D], F32, tag="rs_out", name="rs_out")
                    if USE_COLL:
                        nc.gpsimd.collective_compute(
                            "ReduceScatter", ALU.add,
                            replica_groups=[list(range(N_CORES))],
                            ins=[rs_in_b[:].opt()], outs=[rs_out_b[:].opt()],
                        )
                    else:
                        nc.sync.dma_start(rs_out_b[:], rs_in_b[0:64, :])
                    ag_in_b = ffn_half(rs_out_b)
                    ag_out_b = dp.tile([S, D], F32, tag="ag_out", name="ag_out",
                                       addr_space="Shared")
                    if USE_COLL:
                        nc.gpsimd.collective_compute(
                            "AllGather", ALU.bypass,
                            replica_groups=[list(range(N_CORES))],
                            ins=[ag_in_b[:].opt()], outs=[ag_out_b[:].opt()],
                        )
                    else:
                        for _r in range(N_CORES):
                            nc.sync.dma_start(ag_out_b[_r * 64:(_r + 1) * 64, :], ag_in_b[:])
                    # next-layer residual stream, this batch's 4 token tiles
                    nc.sync.dma_start(
                        x_next[:, b * 4:(b + 1) * 4, :],
                        ag_out_b[:].rearrange("(q p2 o) d -> (p2 o) q d", q=4, p2=2, o=64))

                x_sb = x_next

            # ================ vocab projection ================
            if not DO_VOCAB:
                fin = ap.tile([128, 8], F32, tag="fin")
                nc.vector.tensor_copy(fin[:], x_sb[:, 0, 0:8])
                nc.sync.dma_start(out_d[0:128, 0:8], fin[:])
            xvT = xp.tile([128, KT, T], BF16, tag="zT", name="xvT")
            for t in (range(NT) if DO_VOCAB else []):
                xc = ap.tile([128, D], BF16, tag="z", name="xc")
                nc.vector.tensor_copy(xc[:], x_sb[:, t, :])
                tp = ps_tp.tile([128, KT, 128], BF16, tag="tp", name="tp")
                for k in range(KT):
                    nc.tensor.matmul(tp[:, k, :], xc[:, k * 128:(k + 1) * 128], ident[:],
                                     is_transpose=True, start=(k == 0), stop=(k == KT - 1))
                nc.vector.tensor_copy(xvT[:, :, t * 128:(t + 1) * 128], tp[:])
            for nch in (range(NCHV) if DO_VOCAB else []):
                ow_c = wq.tile([128, KT, VCH], BF16, tag="outwc", name="ow_c")
                nc.sync.dma_start(
                    ow_c[:], outw_d[:, :, nch * VCH:(nch + 1) * VCH].rearrange("k p c -> p k c"))
                ob_c = wq.tile([128, VCH], BF16, tag="outbc", name="ob_c")
                nc.sync.dma_start(ob_c[0:1, :], outb_d[:, nch * VCH:(nch + 1) * VCH])
                for m in range(NT):
                    vp = ps_mm.tile([128, VCH], F32, tag="mm", name="vp")
                    for k in range(KT):
                        nc.tensor.matmul(vp[:], xvT[:, k, m * 128:(m + 1) * 128],
                                         ow_c[:, k, :],
                                         start=(k == 0), stop=False)
                    nc.tensor.matmul(vp[:], ones[0:1, :128], ob_c[0:1, :],
                                     start=False, stop=True)
                    ob = ap.tile([128, VCH], F32, tag="ob", name="ob")
                    if (m * NCHV + nch) % 2 == 0:
                        nc.vector.tensor_copy(ob[:], vp[:])
                    else:
                        nc.scalar.copy(ob[:], vp[:])
                    nc.sync.dma_start(out_d[m * 128:(m + 1) * 128, nch * VCH:(nch + 1) * VCH],
                                      ob[:])

    nc.compile()
    return nc


def prep_inputs(input_ids, token_emb, pos_emb, layers, out_w, out_b):
    """Host-side input marshalling: per-core weight slices, LN folding, layout."""
    bf16 = ml_dtypes.bfloat16
    f32 = np.float32
    ids = np.asarray(input_ids).reshape(-1).astype(np.int64)       # [1024]
    temb = np.asarray(token_emb, f32)
    pos = np.asarray(pos_emb, f32)[:S]
    ident = np.eye(128, dtype=bf16)
    out_w = np.asarray(out_w, f32)
    out_b = np.asarray(out_b, f32)

    vsh = VOCAB // N_CORES
    in_maps = []
    for r in range(N_CORES):
        idl = ids - r * vsh
        idl = np.where((idl >= 0) & (idl < vsh), idl, vsh).astype(np.int32)
        ids_pt = idl.reshape(NT, 128).T.copy()                      # [128, NT]
        temb_sh = np.concatenate([temb[r * vsh:(r + 1) * vsh],
                                  np.zeros((1, D), f32)], axis=0).astype(bf16)
        m = dict(ids=ids_pt, temb=temb_sh, pos=pos, ident=ident)
        for l, p in enumerate(layers):
            qw = np.asarray(p["qw"], f32) * np.asarray(p["ln1g"], f32)[:, None]
            kw = np.asarray(p["kw"], f32) * np.asarray(p["ln1g"], f32)[:, None]
            vw = np.asarray(p["vw"], f32) * np.asarray(p["ln1g"], f32)[:, None]
            qb = np.asarray(p["qb"], f32) + np.asarray(p["ln1b"], f32) @ np.asarray(p["qw"], f32)
            kb = np.asarray(p["kb"], f32) + np.asarray(p["ln1b"], f32) @ np.asarray(p["kw"], f32)
            vb = np.asarray(p["vb"], f32) + np.asarray(p["ln1b"], f32) @ np.asarray(p["vw"], f32)
            hsl = slice(r * HEAD, (r + 1) * HEAD)
            qkvw = np.concatenate([qw[:, hsl], kw[:, hsl], vw[:, hsl]], axis=1)  # [512,192]
            m[f"qkvw{l}"] = qkvw.astype(bf16).reshape(KT, 128, 3 * HEAD)
            m[f"qkvb{l}"] = np.concatenate([qb[hsl], kb[hsl], vb[hsl]])[None, :].astype(bf16)
            m[f"ow{l}"] = np.asarray(p["ow"], f32)[hsl, :].astype(bf16)
            m[f"ob8{l}"] = (np.asarray(p["ob"], f32) / N_CORES)[None, :].astype(bf16)
            c2 = (np.cos(np.asarray(p["phase"], f32)[:, r]) ** 2) * np.float32(1.0 / np.sqrt(HEAD))
            m[f"c2s{l}"] = np.tile(c2[None, :], (128, 1)).astype(f32)
            iw = np.asarray(p["iw"], f32) / np.float32(W)
            m[f"iw16{l}"] = np.tile(iw[None, :], (128, 1)).astype(f32)
            f1w = np.asarray(p["f1w"], f32) * np.asarray(p["ln2g"], f32)[:, None]
            f1b = np.asarray(p["f1b"], f32) + np.asarray(p["ln2b"], f32) @ np.asarray(p["f1w"], f32)
            m[f"f1w{l}"] = f1w.astype(bf16).reshape(KT, 128, FF)
            m[f"f1b{l}"] = f1b[None, :].astype(bf16)
            m[f"f2w{l}"] = np.asarray(p["f2w"], f32).astype(bf16).reshape(KF, 128, D)
            m[f"f2b{l}"] = np.asarray(p["f2b"], f32)[None, :].astype(bf16)
        vsl = slice(r * VS, (r + 1) * VS)
        m["outw"] = out_w[:, vsl].astype(bf16).reshape(KT, 128, VS)
        m["outb"] = out_b[vsl][None, :].astype(bf16)
        in_maps.append(m)
    return in_maps


def _get_exec():
    if "exec" in _CACHE:
        return _CACHE["exec"]
    import jax
    from jax.sharding import Mesh, PartitionSpec
    from jax.experimental.shard_map import shard_map
    from concourse import bass2jax, mybir

    nc = _build_nc()
    bass2jax.install_neuronx_cc_hook()
    partition_name = nc.partition_id_tensor.name if nc.partition_id_tensor else None
    in_names, out_names, out_avals, zero_outs = [], [], [], []
    for alloc in nc.m.functions[0].allocations:
        if not isinstance(alloc, mybir.MemoryLocationSet):
            continue
        name = alloc.memorylocations[0].name
        if alloc.kind == "ExternalInput":
            if name != partition_name:
                in_names.append(name)
        elif alloc.kind == "ExternalOutput":
            out_names.append(name)
            shape = tuple(alloc.tensor_shape)
            dt = mybir.dt.np(alloc.dtype)
            out_avals.append(jax.core.ShapedArray(shape, dt))
            zero_outs.append(np.zeros(shape, dt))
    n_params = len(in_names)
    all_names = list(in_names) + list(out_names) + ([partition_name] if partition_name else [])

    def _body(*args):
        operands = list(args)
        if partition_name is not None:
            operands.append(bass2jax.partition_id_tensor())
        outs = bass2jax._bass_exec_p.bind(
            *operands,
            out_avals=tuple(out_avals),
            in_names=tuple(all_names),
            out_names=tuple(out_names),
            lowering_input_output_aliases=(),
            sim_require_finite=True,
            sim_require_nnan=True,
            nc=nc,
        )
        return tuple(outs)

    devices = jax.devices()[:N_CORES]
    mesh = Mesh(np.asarray(devices), ("core",))
    in_specs = (PartitionSpec("core"),) * (n_params + len(out_names))
    out_specs = (PartitionSpec("core"),) * len(out_names)
    fn = jax.jit(
        shard_map(_body, mesh=mesh, in_specs=in_specs, out_specs=out_specs,
                  check_rep=False),
        keep_unused=True,
    )
    _CACHE["exec"] = (fn, in_names, out_names, out_avals, zero_outs)
    return _CACHE["exec"]


def _device_args(in_maps):
    import jax
    fn, in_names, out_names, out_avals, zero_outs = _get_exec()
    concat_in = [
        np.concatenate([np.asarray(in_maps[c][n]) for c in range(N_CORES)], axis=0)
        for n in in_names
    ]
    concat_zero = [
        np.zeros((N_CORES * z.shape[0],) + z.shape[1:], z.dtype) for z in zero_outs
    ]
    args = [jax.device_put(a) for a in concat_in + concat_zero]
    return fn, out_names, out_avals, args


def kernel(input_ids, token_emb, pos_emb, layers, out_w, out_b):
    in_maps = prep_inputs(input_ids, token_emb, pos_emb, layers, out_w, out_b)
    fn, out_names, out_avals, args = _device_args(in_maps)
    outs = fn(*args)
    out = np.asarray(outs[0])                      # [8*1024, 4000]
    out = out.reshape(N_CORES, T, VS)
    full = np.concatenate([out[c] for c in range(N_CORES)], axis=1)  # [1024, 32000]
    return full.reshape(B, S, VOCAB).astype(np.float32)


# revision 24
# speedup vs baseline: 78.9082x; 1.0220x over previous
"""Photonic transformer on 8 Trainium2 NeuronCores.

Sharding: tensor-parallel attention (1 head per core, all 16 wavelengths
local), token-parallel FFN (two 64-token half-shards per core), vocab-parallel
output projection (4000 cols per core). Comm per layer: per-batch
ReduceScatter of (out-proj partials + x/8) and per-batch AllGather of the
updated residual half — batch 0's RS/FFN/AG overlaps batch 1's attention, and
the per-batch-half LN1/QKV lets layer l+1's batch-0 attention overlap layer
l's tail (cross-layer pipeline). The wavelength softmax uses one ACT exp per
wavelength (per-partition scale = cos^2(phase)/sqrt(64), free-axis accum_out
for row sums), a DVE scale per wavelength and a pairwise tree-add reduction;
the summed attention matrix hits the AV matmul once. The
Kerr factor cos(2.7e-12*s^2) == 1.0 in fp32 and is dropped. LayerNorm affine
params are folded into the following weights on the host.
"""
import os
import numpy as np
import ml_dtypes

N_CORES = 8
B, S, D, H, HEAD, W, L, VOCAB = 2, 512, 512, 8, 64, 16, 6, 32000
T = B * S              # 1024 tokens
NT = T // 128          # 8 token tiles
KT = D // 128          # 4 k-tiles over D
FF = 4 * D             # 2048
KF = FF // 128         # 16 k-tiles over FF
VS = VOCAB // N_CORES  # 4000 vocab cols per core
NCHV = 8
VCH = VS // NCHV       # 500
LN_EPS = 1e-5

# wavelength indices whose accumulate runs on gpsimd (empirical knob)
GPS_W = ()
USE_TREE = os.environ.get("PK_TREE", "1") == "1"
L_RUN = int(os.environ.get("PK_LAYERS", L))
USE_COLL = os.environ.get("PK_COLL", "1") == "1"
DO_VOCAB = os.environ.get("PK_VOCAB", "1") == "1"

_CACHE = {}


def _build_nc():
    from concourse import bacc, mybir, tile
    from concourse.bass import IndirectOffsetOnAxis

    F32, BF16, I32 = mybir.dt.float32, mybir.dt.bfloat16, mybir.dt.int32
    AF, ALU = mybir.ActivationFunctionType, mybir.AluOpType

    nc = bacc.Bacc(None, num_devices=N_CORES)

    # ---------------- I/O ----------------
    ids_d = nc.dram_tensor("ids", [128, NT], I32, kind="ExternalInput")
    temb_d = nc.dram_tensor("temb", [VOCAB // N_CORES + 1, D], BF16, kind="ExternalInput")
    pos_d = nc.dram_tensor("pos", [S, D], F32, kind="ExternalInput")
    ident_d = nc.dram_tensor("ident", [128, 128], BF16, kind="ExternalInput")
    lw = []
    for l in range(L):
        lw.append(dict(
            qkvw=nc.dram_tensor(f"qkvw{l}", [KT, 128, 3 * HEAD], BF16, kind="ExternalInput"),
            qkvb=nc.dram_tensor(f"qkvb{l}", [1, 3 * HEAD], BF16, kind="ExternalInput"),
            ow=nc.dram_tensor(f"ow{l}", [HEAD, D], BF16, kind="ExternalInput"),
            ob8=nc.dram_tensor(f"ob8{l}", [1, D], BF16, kind="ExternalInput"),
            c2s=nc.dram_tensor(f"c2s{l}", [128, W], F32, kind="ExternalInput"),
            iw16=nc.dram_tensor(f"iw16{l}", [128, W], F32, kind="ExternalInput"),
            f1w=nc.dram_tensor(f"f1w{l}", [KT, 128, FF], BF16, kind="ExternalInput"),
            f1b=nc.dram_tensor(f"f1b{l}", [1, FF], BF16, kind="ExternalInput"),
            f2w=nc.dram_tensor(f"f2w{l}", [KF, 128, D], BF16, kind="ExternalInput"),
            f2b=nc.dram_tensor(f"f2b{l}", [1, D], BF16, kind="ExternalInput"),
        ))
    outw_d = nc.dram_tensor("outw", [KT, 128, VS], BF16, kind="ExternalInput")
    outb_d = nc.dram_tensor("outb", [1, VS], BF16, kind="ExternalInput")
    out_d = nc.dram_tensor("out", [T, VS], F32, kind="ExternalOutput")

    with tile.TileContext(nc) as tc:
        with (
            tc.tile_pool(name="const", bufs=1) as cpool,
            tc.tile_pool(name="wq", bufs=2) as wq,        # per-layer small weights
            tc.tile_pool(name="wf", bufs=1) as wf,        # per-layer FFN weights
            tc.tile_pool(name="xres", bufs=2) as xres,    # residual stream (x)
            tc.tile_pool(name="xp", bufs=1) as xp,        # per-layer persistents
            tc.tile_pool(name="act", bufs=2) as ap,       # activations
            tc.tile_pool(name="ep", bufs=2) as ep,        # exp tiles
            tc.tile_pool(name="sm", bufs=2) as sm,        # small stats
            tc.tile_pool(name="dram", bufs=2, space="DRAM") as dp,
            tc.tile_pool(name="ps_sc", bufs=1, space="PSUM") as ps_sc,
            tc.tile_pool(name="ps_mm", bufs=2, space="PSUM") as ps_mm,
            tc.tile_pool(name="ps_tp", bufs=2, space="PSUM") as ps_tp,
            tc.tile_pool(name="ps_h1", bufs=1, space="PSUM") as ps_h1,
        ):
            ones = cpool.tile([128, T], BF16)
            nc.gpsimd.memset(ones[:], 1.0)
            ident = cpool.tile([128, 128], BF16)
            nc.sync.dma_start(ident[:], ident_d[:])
            ids_sb = cpool.tile([128, NT], I32)
            nc.sync.dma_start(ids_sb[:], ids_d[:])
            pos_sb = ep.tile([128, NT // 2, D], F32, tag="E")
            for t in range(NT // 2):
                nc.sync.dma_start(pos_sb[:, t, :], pos_d[t * 128:(t + 1) * 128, :])

            # ---------------- embedding (vocab-sharded gather + AllReduce) ----
            x_sb = xres.tile([128, NT, D], F32, tag="x")
            gath = ep.tile([128, NT, D], BF16, tag="E")
            nc.gpsimd.memset(gath[:], 0.0)
            for t in range(NT):
                nc.gpsimd.indirect_dma_start(
                    out=gath[:, t, :], out_offset=None,
                    in_=temb_d[:, :],
                    in_offset=IndirectOffsetOnAxis(ap=ids_sb[:, t:t + 1], axis=0),
                    bounds_check=VOCAB // N_CORES - 1, oob_is_err=False,
                )
            emb_in = dp.tile([T, D], BF16, tag="emb_in")
            emb_out = dp.tile([T, D], BF16, tag="emb_out", addr_space="Shared")
            for t in range(NT):
                nc.sync.dma_start(emb_in[t * 128:(t + 1) * 128, :], gath[:, t, :])
            if USE_COLL:
                nc.gpsimd.collective_compute(
                    "AllReduce", ALU.add,
                    replica_groups=[list(range(N_CORES))],
                    ins=[emb_in[:].opt()], outs=[emb_out[:].opt()],
                )
            else:
                nc.sync.dma_start(emb_out[:], emb_in[:])
            for t in range(NT):
                nc.sync.dma_start(gath[:, t, :], emb_out[t * 128:(t + 1) * 128, :])
            for t in range(NT):
                nc.vector.tensor_tensor(x_sb[:, t, :], gath[:, t, :],
                                        pos_sb[:, t % (NT // 2), :], ALU.add)

            def quake_rstd(veps, n, rows=128):
                """veps [rows, n] f32 AP (var+eps) -> rstd tile, quake + 3 Newton."""
                y0i_t = sm.tile([128, n], I32, tag="qk_i", name="qk_i")
                y0i = y0i_t[0:rows]
                nc.vector.tensor_scalar(y0i, veps.bitcast(I32), 1, None,
                                        op0=ALU.logical_shift_right)
                nc.vector.tensor_scalar(y0i, y0i, 0x5F3759DF, -1,
                                        op0=ALU.subtract, op1=ALU.mult)
                yt = sm.tile([128, n], F32, tag="qk_y", name="qk_y")
                y = yt[0:rows]
                nc.vector.tensor_copy(y, y0i.bitcast(F32))
                t1_t = sm.tile([128, n], F32, tag="qk_t", name="qk_t")
                t1 = t1_t[0:rows]
                for _ in range(3):
                    nc.vector.tensor_tensor(t1, y, y, ALU.mult)
                    nc.vector.tensor_tensor(t1, t1, veps, ALU.mult)
                    nc.vector.tensor_scalar(t1, t1, -0.5, 1.5, op0=ALU.mult, op1=ALU.add)
                    nc.vector.tensor_tensor(y, y, t1, ALU.mult)
                return yt

            # ================ layers ================
            for l in range(L_RUN):
                w = lw[l]
                qkvw = wq.tile([128, KT, 3 * HEAD], BF16, tag="qkvw")
                qkvb = wq.tile([128, 3 * HEAD], BF16, tag="qkvb")
                ow_sb = wq.tile([128, D], BF16, tag="ow")
                ob8 = wq.tile([128, D], BF16, tag="ob8")
                c2s = wq.tile([128, W], F32, tag="c2s")
                iw16 = wq.tile([128, W], F32, tag="iw16")
                nc.sync.dma_start(qkvw[:], w["qkvw"][:].rearrange("k p c -> p k c"))
                nc.sync.dma_start(qkvb[0:1, :], w["qkvb"][:])
                nc.sync.dma_start(ow_sb[0:HEAD, :], w["ow"][:])
                nc.sync.dma_start(ob8[0:1, :], w["ob8"][:])
                nc.sync.dma_start(c2s[:], w["c2s"][:])
                nc.sync.dma_start(iw16[:], w["iw16"][:])
                f1w = wf.tile([128, KT, FF], BF16, tag="f1w")
                f1b = wf.tile([128, FF], BF16, tag="f1b")
                f2w = wf.tile([128, KF, D], BF16, tag="f2w")
                f2b = wf.tile([128, D], BF16, tag="f2b")
                nc.sync.dma_start(f1w[:], w["f1w"][:].rearrange("k p c -> p k c"))
                nc.sync.dma_start(f1b[0:1, :], w["f1b"][:])
                nc.sync.dma_start(f2w[:], w["f2w"][:].rearrange("k p c -> p k c"))
                nc.sync.dma_start(f2b[0:1, :], w["f2b"][:])

                zT = xp.tile([128, KT, T], BF16, tag="zT")
                qT = ap.tile([128, T], BF16, tag="qT")
                kTt = ap.tile([128, T], BF16, tag="kTt")
                v_sb = xp.tile([128, NT, HEAD], BF16, tag="v_sb")
                OT_sb = xp.tile([128, NT, 128], BF16, tag="OT")
                x_next = xres.tile([128, NT, D], F32, tag="x", name="x_next")

                def ffn_half(rs_out_b):
                    """LN2 + photonic FFN on this core's 64-token half."""
                    x1 = ap.tile([128, D], F32, tag="x1h", name="x1h")
                    nc.sync.dma_start(x1[0:64, :], rs_out_b[:])
                    st6b = sm.tile([128, 1, 6], F32, tag="st6b", name="st6b")
                    mvb = sm.tile([128, 1, 2], F32, tag="mvb", name="mvb")
                    nc.vector.bn_stats(st6b[0:64, 0, :], x1[0:64, :])
                    nc.vector.bn_aggr(mvb[0:64, 0, :], st6b[0:64, 0, :])
                    negmu2 = sm.tile([128, 1], F32, tag="negmu2", name="negmu2")
                    veps2 = sm.tile([128, 1], F32, tag="veps2", name="veps2")
                    nc.vector.tensor_scalar_mul(negmu2[0:64], mvb[0:64, :, 0], -1.0)
                    nc.vector.tensor_scalar_add(veps2[0:64], mvb[0:64, :, 1], LN_EPS)
                    rstd2 = quake_rstd(veps2[0:64], 1, rows=64)
                    z2 = ap.tile([128, D], BF16, tag="z2", name="z2")
                    nc.vector.tensor_scalar(z2[0:64, :], x1[0:64, :],
                                            negmu2[0:64, 0:1], rstd2[0:64, 0:1],
                                            op0=ALU.add, op1=ALU.mult)
                    z2T = ap.tile([128, KT, 64], BF16, tag="z2T", name="z2T")
                    tp = ps_tp.tile([128, KT, 128], BF16, tag="tp", name="tp")
                    for k in range(KT):
                        nc.tensor.matmul(tp[:, k, 0:64], z2[0:64, k * 128:(k + 1) * 128],
                                         ident[0:64, 0:64], is_transpose=True,
                                         start=(k == 0), stop=(k == KT - 1))
                    nc.vector.tensor_copy(z2T[:], tp[:, :, 0:64])
                    actF = ap.tile([128, KF, 64], BF16, tag="actF", name="actF")
                    for hf in range(2):
                        h1 = ps_h1.tile([128, 2, 512], F32, tag="h1", name="h1")
                        for nch in range(2):
                            ncol = hf * 1024 + nch * 512
                            for k in range(KT):
                                nc.tensor.matmul(h1[0:64, nch, :], z2T[:, k, :],
                                                 f1w[:, k, ncol:ncol + 512],
                                                 start=(k == 0), stop=False)
                            nc.tensor.matmul(h1[0:64, nch, :], ones[0:1, 0:64],
                                             f1b[0:1, ncol:ncol + 512], start=False, stop=True)
                        for nch in range(2):
                            tnh = ap.tile([128, 512], BF16, tag="tnh", name="tnh")
                            nc.scalar.activation(tnh[0:64, :], h1[0:64, nch, :], AF.Tanh,
                                                 scale=2.0)
                            den = ap.tile([128, 512], F32, tag="den", name="den")
                            nc.scalar.activation(den[0:64, :], h1[0:64, nch, :], AF.Abs)
                            nc.vector.tensor_scalar_add(den[0:64, :], den[0:64, :], 1.0)
                            nc.vector.reciprocal_approx_fast(den[0:64, :], den[0:64, :])
                            af = ap.tile([128, 512], BF16, tag="af", name="af")
                            nc.vector.scalar_tensor_tensor(af[0:64, :], den[0:64, :], 1.0,
                                                           tnh[0:64, :],
                                                           op0=ALU.mult, op1=ALU.mult)
                            tp = ps_tp.tile([128, KT, 128], BF16, tag="tp", name="tp")
                            for j in range(4):
                                nc.tensor.matmul(tp[:, j, 0:64],
                                                 af[0:64, j * 128:(j + 1) * 128],
                                                 ident[0:64, 0:64], is_transpose=True,
                                                 start=(j == 0), stop=(j == 3))
                            nc.vector.tensor_copy(
                                actF[:, hf * 8 + nch * 4:hf * 8 + nch * 4 + 4, :],
                                tp[:, :, 0:64])
                    x2 = ps_mm.tile([128, D], F32, tag="mm", name="x2")
                    for kk in range(KF):
                        nc.tensor.matmul(x2[0:64, :], actF[:, kk, :], f2w[:, kk, :],
                                         start=(kk == 0), stop=False)
                    nc.tensor.matmul(x2[0:64, :], ones[0:1, 0:64], f2b[0:1, :],
                                     start=False, stop=True)
                    y_sb = ap.tile([128, D], F32, tag="y_sb", name="y_sb")
                    nc.vector.scalar_tensor_tensor(y_sb[0:64, :], x1[0:64, :], 1.0,
                                                   x2[0:64, :], op0=ALU.mult, op1=ALU.add)
                    ag_in = dp.tile([64, D], F32, tag="ag_in", name="ag_in")
                    nc.sync.dma_start(ag_in[:], y_sb[0:64, :])
                    return ag_in

                for b in range(B):
                    bsl = slice(b * 512, (b + 1) * 512)
                    # ---- LN1 + z^T + QKV for this batch half
                    st6 = sm.tile([128, 4, 6], F32, tag="st6", name="st6")
                    mv = sm.tile([128, 4, 2], F32, tag="mv", name="mv")
                    for q in range(4):
                        t = b * 4 + q
                        nc.vector.bn_stats(st6[:, q, :], x_sb[:, t, :])
                        nc.vector.bn_aggr(mv[:, q, :], st6[:, q, :])
                    negmu = sm.tile([128, 4], F32, tag="negmu", name="negmu")
                    veps = sm.tile([128, 4], F32, tag="veps", name="veps")
                    nc.vector.tensor_scalar_mul(negmu[:], mv[:, :, 0], -1.0)
                    nc.vector.tensor_scalar_add(veps[:], mv[:, :, 1], LN_EPS)
                    rstd = quake_rstd(veps[:], 4)
                    for q in range(4):
                        t = b * 4 + q
                        z = ap.tile([128, D], BF16, tag="z", name="z")
                        nc.vector.tensor_scalar(z[:], x_sb[:, t, :],
                                                negmu[:, q:q + 1], rstd[:, q:q + 1],
                                                op0=ALU.add, op1=ALU.mult)
                        tp = ps_tp.tile([128, KT, 128], BF16, tag="tp", name="tp")
                        for k in range(KT):
                            nc.tensor.matmul(tp[:, k, :], z[:, k * 128:(k + 1) * 128],
                                             ident[:], is_transpose=True,
                                             start=(k == 0), stop=(k == KT - 1))
                        nc.vector.tensor_copy(zT[:, :, t * 128:(t + 1) * 128], tp[:])
                    for which, dst in ((0, qT), (1, kTt)):
                        ps_full = ps_mm.tile([128, 512], F32, tag="mm", name="qk_ps")
                        ps = ps_full[0:HEAD, :]
                        cs = slice(which * HEAD, (which + 1) * HEAD)
                        for k in range(KT):
                            nc.tensor.matmul(ps, qkvw[:, k, cs], zT[:, k, bsl],
                                             start=(k == 0), stop=False)
                        nc.tensor.matmul(ps, qkvb[0:1, cs], ones[0:1, bsl],
                                         start=False, stop=True)
                        nc.vector.tensor_copy(dst[0:HEAD, bsl], ps)
                    for q in range(4):
                        t = b * 4 + q
                        ps = ps_mm.tile([128, HEAD], F32, tag="mm", name="v_ps")
                        for k in range(KT):
                            nc.tensor.matmul(ps[:], zT[:, k, t * 128:(t + 1) * 128],
                                             qkvw[:, k, 2 * HEAD:3 * HEAD],
                                             start=(k == 0), stop=False)
                        nc.tensor.matmul(ps[:], ones[0:1, :128],
                                         qkvb[0:1, 2 * HEAD:3 * HEAD], start=False, stop=True)
                        nc.vector.tensor_copy(v_sb[:, t, :], ps[:])

                    # ---- attention over the batch
                    rs_in_b = dp.tile([S, D], F32, tag="rs_in", name="rs_in")
                    AT = ap.tile([128, KT, 512], BF16, tag="AT", name="AT")
                    for st in range(4):
                        sc = ps_sc.tile([128, 512], F32, tag="sc", name="sc")
                        nc.tensor.matmul(
                            sc[:], qT[0:HEAD, b * 512 + st * 128: b * 512 + (st + 1) * 128],
                            kTt[0:HEAD, bsl])
                        E = ep.tile([128, W, 512], BF16, tag="E", name="E")
                        Z = sm.tile([128, W], F32, tag="Z", name="Z")
                        for wl in range(W):
                            nc.scalar.activation(E[:, wl, :], sc[:], AF.Exp,
                                                 scale=c2s[:, wl:wl + 1],
                                                 accum_out=Z[:, wl:wl + 1])
                        R = sm.tile([128, W], F32, tag="R", name="R")
                        nc.vector.reciprocal_approx_fast(R[:], Z[:])
                        nc.vector.tensor_tensor(R[:], R[:], iw16[:], ALU.mult)
                        A = ap.tile([128, 512], BF16, tag="A", name="A")
                        if USE_TREE:
                            # P_w = E_w * r_w in place, then pairwise tree adds
                            for wl in range(W):
                                nc.vector.tensor_scalar_mul(E[:, wl, :], E[:, wl, :],
                                                            R[:, wl:wl + 1])
                            nc.vector.tensor_tensor(E[:, 0:8, :], E[:, 0:8, :],
                                                    E[:, 8:16, :], ALU.add)
                            nc.vector.tensor_tensor(E[:, 0:4, :], E[:, 0:4, :],
                                                    E[:, 4:8, :], ALU.add)
                            nc.vector.tensor_tensor(E[:, 0:2, :], E[:, 0:2, :],
                                                    E[:, 2:4, :], ALU.add)
                            nc.vector.tensor_tensor(A[:], E[:, 0, :], E[:, 1, :], ALU.add)
                        else:
                            nc.vector.tensor_scalar_mul(A[:], E[:, 0, :], R[:, 0:1])
                            for wl in range(1, W):
                                nc.vector.scalar_tensor_tensor(
                                    A[:], E[:, wl, :], R[:, wl:wl + 1], A[:],
                                    op0=ALU.mult, op1=ALU.add)
                        tp = ps_tp.tile([128, KT, 128], BF16, tag="tp", name="tp")
                        for j in range(KT):
                            nc.tensor.matmul(tp[:, j, :], A[:, j * 128:(j + 1) * 128],
                                             ident[:], is_transpose=True,
                                             start=(j == 0), stop=(j == KT - 1))
                        nc.vector.tensor_copy(AT[:, :, st * 128:(st + 1) * 128], tp[:])
                        tok = b * 4 + st
                        ot_f = ps_mm.tile([128, 128], F32, tag="mm", name="ot_f")
                        for j in range(KT):
                            nc.tensor.matmul(ot_f[0:HEAD, :],
                                             v_sb[:, b * 4 + j, :],
                                             AT[:, j, st * 128:(st + 1) * 128],
                                             start=(j == 0), stop=(j == KT - 1))
                        nc.vector.tensor_copy(OT_sb[0:HEAD, tok, :], ot_f[0:HEAD, :])
                        pp = ps_mm.tile([128, D], F32, tag="mm", name="pp")
                        nc.tensor.matmul(pp[:], OT_sb[0:HEAD, tok, :], ow_sb[0:HEAD, :],
                                         start=True, stop=False)
                        nc.tensor.matmul(pp[:], ones[0:1, :128], ob8[0:1, :],
                                         start=False, stop=True)
                        p_sb = ap.tile([128, D], F32, tag="p_sb", name="p_sb")
                        nc.vector.scalar_tensor_tensor(p_sb[:], x_sb[:, tok, :], 0.125,
                                                       pp[:], op0=ALU.mult, op1=ALU.add)
                        nc.sync.dma_start(rs_in_b[st * 128:(st + 1) * 128, :], p_sb[:])

                    # ---- per-batch ReduceScatter -> half FFN -> per-batch AllGather
                    rs_out_b = dp.tile([64, D], F32, tag="rs_out", name="rs_out")
                    if USE_COLL:
                        nc.gpsimd.collective_compute(
                            "ReduceScatter", ALU.add,
                            replica_groups=[list(range(N_CORES))],
                            ins=[rs_in_b[:].opt()], outs=[rs_out_b[:].opt()],
                        )
                    else:
                        nc.sync.dma_start(rs_out_b[:], rs_in_b[0:64, :])
                    ag_in_b = ffn_half(rs_out_b)
                    ag_out_b = dp.tile([S, D], F32, tag="ag_out", name="ag_out",
                                       addr_space="Shared")
                    if USE_COLL:
                        nc.gpsimd.collective_compute(
                            "AllGather", ALU.bypass,
                            replica_groups=[list(range(N_CORES))],
                            ins=[ag_in_b[:].opt()], outs=[ag_out_b[:].opt()],
                        )
                    else:
                        for _r in range(N_CORES):
                            nc.sync.dma_start(ag_out_b[_r * 64:(_r + 1) * 64, :], ag_in_b[:])
                    # next-layer residual stream, this batch's 4 token tiles
                    nc.sync.dma_start(
                        x_next[:, b * 4:(b + 1) * 4, :],
                        ag_out_b[:].rearrange("(q p2 o) d -> (p2 o) q d", q=4, p2=2, o=64))

                x_sb = x_next

            # ================ vocab projection ================
            if not DO_VOCAB:
                fin = ap.tile([128, 8], F32, tag="fin")
                nc.vector.tensor_copy(fin[:], x_sb[:, 0, 0:8])
                nc.sync.dma_start(out_d[0:128, 0:8], fin[:])
            xvT = xp.tile([128, KT, T], BF16, tag="zT", name="xvT")
            for t in (range(NT) if DO_VOCAB else []):
                xc = ap.tile([128, D], BF16, tag="z", name="xc")
                nc.vector.tensor_copy(xc[:], x_sb[:, t, :])
                tp = ps_tp.tile([128, KT, 128], BF16, tag="tp", name="tp")
                for k in range(KT):
                    nc.tensor.matmul(tp[:, k, :], xc[:, k * 128:(k + 1) * 128], ident[:],
                                     is_transpose=True, start=(k == 0), stop=(k == KT - 1))
                nc.vector.tensor_copy(xvT[:, :, t * 128:(t + 1) * 128], tp[:])
            for nch in (range(NCHV) if DO_VOCAB else []):
                ow_c = wq.tile([128, KT, VCH], BF16, tag="outwc", name="ow_c")
                nc.sync.dma_start(
                    ow_c[:], outw_d[:, :, nch * VCH:(nch + 1) * VCH].rearrange("k p c -> p k c"))
                ob_c = wq.tile([128, VCH], BF16, tag="outbc", name="ob_c")
                nc.sync.dma_start(ob_c[0:1, :], outb_d[:, nch * VCH:(nch + 1) * VCH])
                for m in range(NT):
                    vp = ps_mm.tile([128, VCH], F32, tag="mm", name="vp")
                    for k in range(KT):
                        nc.tensor.matmul(vp[:], xvT[:, k, m * 128:(m + 1) * 128],
                                         ow_c[:, k, :],
                                         start=(k == 0), stop=False)
                    nc.tensor.matmul(vp[:], ones[0:1, :128], ob_c[0:1, :],
                                     start=False, stop=True)
                    ob = ap.tile([128, VCH], F32, tag="ob", name="ob")
                    if (m * NCHV + nch) % 2 == 0:
                        nc.vector.tensor_copy(ob[:], vp[:])
                    else:
                        nc.scalar.copy(ob[:], vp[:])
                    nc.sync.dma_start(out_d[m * 128:(m + 1) * 128, nch * VCH:(nch + 1) * VCH],
                                      ob[:])

    nc.compile()
    return nc


def prep_inputs(input_ids, token_emb, pos_emb, layers, out_w, out_b):
    """Host-side input marshalling: per-core weight slices, LN folding, layout."""
    bf16 = ml_dtypes.bfloat16
    f32 = np.float32
    ids = np.asarray(input_ids).reshape(-1).astype(np.int64)       # [1024]
    temb = np.asarray(token_emb, f32)
    pos = np.asarray(pos_emb, f32)[:S]
    ident = np.eye(128, dtype=bf16)
    out_w = np.asarray(out_w, f32)
    out_b = np.asarray(out_b, f32)

    vsh = VOCAB // N_CORES
    in_maps = []
    for r in range(N_CORES):
        idl = ids - r * vsh
        idl = np.where((idl >= 0) & (idl < vsh), idl, vsh).astype(np.int32)
        ids_pt = idl.reshape(NT, 128).T.copy()                      # [128, NT]
        temb_sh = np.concatenate([temb[r * vsh:(r + 1) * vsh],
                                  np.zeros((1, D), f32)], axis=0).astype(bf16)
        m = dict(ids=ids_pt, temb=temb_sh, pos=pos, ident=ident)
        for l, p in enumerate(layers):
            qw = np.asarray(p["qw"], f32) * np.asarray(p["ln1g"], f32)[:, None]
            kw = np.asarray(p["kw"], f32) * np.asarray(p["ln1g"], f32)[:, None]
            vw = np.asarray(p["vw"], f32) * np.asarray(p["ln1g"], f32)[:, None]
            qb = np.asarray(p["qb"], f32) + np.asarray(p["ln1b"], f32) @ np.asarray(p["qw"], f32)
            kb = np.asarray(p["kb"], f32) + np.asarray(p["ln1b"], f32) @ np.asarray(p["kw"], f32)
            vb = np.asarray(p["vb"], f32) + np.asarray(p["ln1b"], f32) @ np.asarray(p["vw"], f32)
            hsl = slice(r * HEAD, (r + 1) * HEAD)
            qkvw = np.concatenate([qw[:, hsl], kw[:, hsl], vw[:, hsl]], axis=1)  # [512,192]
            m[f"qkvw{l}"] = qkvw.astype(bf16).reshape(KT, 128, 3 * HEAD)
            m[f"qkvb{l}"] = np.concatenate([qb[hsl], kb[hsl], vb[hsl]])[None, :].astype(bf16)
            m[f"ow{l}"] = np.asarray(p["ow"], f32)[hsl, :].astype(bf16)
            m[f"ob8{l}"] = (np.asarray(p["ob"], f32) / N_CORES)[None, :].astype(bf16)
            c2 = (np.cos(np.asarray(p["phase"], f32)[:, r]) ** 2) * np.float32(1.0 / np.sqrt(HEAD))
            m[f"c2s{l}"] = np.tile(c2[None, :], (128, 1)).astype(f32)
            iw = np.asarray(p["iw"], f32) / np.float32(W)
            m[f"iw16{l}"] = np.tile(iw[None, :], (128, 1)).astype(f32)
            f1w = np.asarray(p["f1w"], f32) * np.asarray(p["ln2g"], f32)[:, None]
            f1b = np.asarray(p["f1b"], f32) + np.asarray(p["ln2b"], f32) @ np.asarray(p["f1w"], f32)
            m[f"f1w{l}"] = f1w.astype(bf16).reshape(KT, 128, FF)
            m[f"f1b{l}"] = f1b[None, :].astype(bf16)
            m[f"f2w{l}"] = np.asarray(p["f2w"], f32).astype(bf16).reshape(KF, 128, D)
            m[f"f2b{l}"] = np.asarray(p["f2b"], f32)[None, :].astype(bf16)
        vsl = slice(r * VS, (r + 1) * VS)
        m["outw"] = out_w[:, vsl].astype(bf16).reshape(KT, 128, VS)
        m["outb"] = out_b[vsl][None, :].astype(bf16)
        in_maps.append(m)
    return in_maps


def _get_exec():
    if "exec" in _CACHE:
        return _CACHE["exec"]
    import jax
    from jax.sharding import Mesh, PartitionSpec
    from jax.experimental.shard_map import shard_map
    from concourse import bass2jax, mybir

    nc = _build_nc()
    bass2jax.install_neuronx_cc_hook()
    partition_name = nc.partition_id_tensor.name if nc.partition_id_tensor else None
    in_names, out_names, out_avals, zero_outs = [], [], [], []
    for alloc in nc.m.functions[0].allocations:
        if not isinstance(alloc, mybir.MemoryLocationSet):
            continue
        name = alloc.memorylocations[0].name
        if alloc.kind == "ExternalInput":
            if name != partition_name:
                in_names.append(name)
        elif alloc.kind == "ExternalOutput":
            out_names.append(name)
            shape = tuple(alloc.tensor_shape)
            dt = mybir.dt.np(alloc.dtype)
            out_avals.append(jax.core.ShapedArray(shape, dt))
            zero_outs.append(np.zeros(shape, dt))
    n_params = len(in_names)
    all_names = list(in_names) + list(out_names) + ([partition_name] if partition_name else [])

    def _body(*args):
        operands = list(args)
        if partition_name is not None:
            operands.append(bass2jax.partition_id_tensor())
        outs = bass2jax._bass_exec_p.bind(
            *operands,
            out_avals=tuple(out_avals),
            in_names=tuple(all_names),
            out_names=tuple(out_names),
            lowering_input_output_aliases=(),
            sim_require_finite=True,
            sim_require_nnan=True,
            nc=nc,
        )
        return tuple(outs)

    devices = jax.devices()[:N_CORES]
    mesh = Mesh(np.asarray(devices), ("core",))
    in_specs = (PartitionSpec("core"),) * (n_params + len(out_names))
    out_specs = (PartitionSpec("core"),) * len(out_names)
    fn = jax.jit(
        shard_map(_body, mesh=mesh, in_specs=in_specs, out_specs=out_specs,
                  check_rep=False),
        keep_unused=True,
    )
    _CACHE["exec"] = (fn, in_names, out_names, out_avals, zero_outs)
    return _CACHE["exec"]


def _device_args(in_maps):
    import jax
    fn, in_names, out_names, out_avals, zero_outs = _get_exec()
    concat_in = [
        np.concatenate([np.asarray(in_maps[c][n]) for c in range(N_CORES)], axis=0)
        for n in in_names
    ]
    concat_zero = [
        np.zeros((N_CORES * z.shape[0],) + z.shape[1:], z.dtype) for z in zero_outs
    ]
    args = [jax.device_put(a) for a in concat_in + concat_zero]
    return fn, out_names, out_avals, args


def kernel(input_ids, token_emb, pos_emb, layers, out_w, out_b):
    in_maps = prep_inputs(input_ids, token_emb, pos_emb, layers, out_w, out_b)
    fn, out_names, out_avals, args = _device_args(in_maps)
    outs = fn(*args)
    out = np.asarray(outs[0])                      # [8*1024, 4000]
    out = out.reshape(N_CORES, T, VS)
    full = np.concatenate([out[c] for c in range(N_CORES)], axis=1)  # [1024, 32000]
    return full.reshape(B, S, VOCAB).astype(np.float32)
